# revision 31
# baseline (speedup 1.0000x reference)
"""Trainium2 Bass kernel for 2-layer GAT (nn_GAT_34832184770812).

Strategy (8 NeuronCores, dst-node sharded):
- Each core owns 1250 dst nodes; node ids are rotated per core so own nodes
  are local rows 0:1250 (keeps the SPMD program identical across cores).
- Phase A: T1 = features @ [W1 | W1@al1 | W1@ar1] (bf16, replicated) ->
  DRAM gather table T1tab[N, 384] (feat 256 | el 256:260 | er 260:264 | pad).
- Phase B (layer-1 edge phase): edges sorted by dst window (128 dst rows per
  window, padded to 128-edge chunks, chunk counts uniform across cores).
  Per 1024-edge superchunk: one dma_gather of src rows (Pool/SWDGE — no
  HWDGE); one batched load of the one-hot dst indicators for all 8 chunks;
  er per edge via indicator matmuls; e = lrelu(el+er), ex = exp(e) (softmax
  is shift-invariant, exponents are O(1)); messages scaled by ex; segment
  sum over dst via indicator-transpose matmul (IndT resident in SBUF), with
  ex as extra columns producing softmax denominators in the same psum.
- Window finalize: normalize, ELU, transpose (PE), T2own = h @ W2p.
- AllGather T2own (bf16, 52-wide) -> expand to 256B rows -> T2all gather
  table.
- Phase D (layer-2 edge phase): same structure, 1 head, 47 feats.
- log_softmax per window, output [1250, 47] f32 per core, host concat.

DMA-issue budget drives the design: every HWDGE dma_start costs ~630ns on a
device shared by all queues, so host-constant tables (indices, weights,
indicator transposes) are loaded once outside the rep loop, per-chunk loads
are batched per superchunk, and gathers/bulk copies ride the Pool-engine
SWDGE path which bypasses HWDGE entirely.
"""

import numpy as np
import ml_dtypes

BF16 = ml_dtypes.bfloat16

# problem constants (hardcoded per contract)
N = 10000
E = 320000
IN_FEATS = 256
H = 4
D = 64
HD = 256
OUTF = 47
NEG = 0.2
NCORES = 8
OWN = N // NCORES          # 1250
P = 128
NWIN = (OWN + P - 1) // P  # 10 windows (last has 98 nodes)
WIN_SIZES = [min(P, OWN - P * w) for w in range(NWIN)]
K = 16                     # chunks per superchunk
KP = K * P                 # edges per superchunk
IW = KP // 16              # idx cols per superchunk (wrapped 16-row layout)
ROW1 = 512                 # T1 gather row (fp8): feat 0:256 | el bf16 @bytes 256:264 | er bf16 @bytes 264:272 | pad
ROW2 = 128                 # T2 gather row (bf16): feat 0:47 | el2 47 | er2 48 | ex2 49 | pad
T2W = 52                   # t2own row width (47 feat + el + er + ex + pad to 52)
T2OWN_ROWS = NWIN * P      # 1280 (rows 1250:1280 zeroed)

_CACHE = {}


# ----------------------------------------------------------------------------
# host-side graph preprocessing
# ----------------------------------------------------------------------------

def _prep_graph(src, dst):
    """Per-core edge partition, window sort, uniform padding, one-hots."""
    src = np.asarray(src).astype(np.int64)
    dst = np.asarray(dst).astype(np.int64)
    core_of = dst // OWN
    per_core = []
    for c in range(NCORES):
        sel = np.nonzero(core_of == c)[0]
        dl = dst[sel] - OWN * c                       # local dst in [0, OWN)
        sl = (src[sel] - OWN * c) % N                 # local src
        # sort by (window, src) — src-ascending gathers get HBM locality;
        # dstrow within a chunk is free (one-hots encode it)
        order = np.lexsort((sl, dl // P))
        per_core.append((dl[order], sl[order], src[sel][order]))

    # uniform chunks per window across cores
    cw = []
    bounds = []
    for c in range(NCORES):
        dl = per_core[c][0]
        b = np.searchsorted(dl, [P * w for w in range(NWIN + 1)])
        bounds.append(b)
    for w in range(NWIN):
        mx = max(bounds[c][w + 1] - bounds[c][w] for c in range(NCORES))
        cw.append((int(mx) + P - 1) // P)
    nchunk = sum(cw)
    nsc = (nchunk + K - 1) // K
    pad_chunks = nsc * K - nchunk
    cw[-1] += pad_chunks
    nchunk = nsc * K

    chunk_win = []
    for w in range(NWIN):
        chunk_win += [w] * cw[w]

    ES = nchunk * P
    out = []
    for c in range(NCORES):
        dl, sl, sg = per_core[c]
        b = bounds[c]
        src_loc = np.zeros(ES, np.int16)
        src_glb = np.zeros(ES, np.int16)
        dstrow = np.full(ES, -1, np.int32)            # -1 = dummy
        pos = 0
        for w in range(NWIN):
            e0, e1 = b[w], b[w + 1]
            n = e1 - e0
            src_loc[pos : pos + n] = sl[e0:e1]
            src_glb[pos : pos + n] = sg[e0:e1]
            dstrow[pos : pos + n] = dl[e0:e1] - P * w
            pos += cw[w] * P
        # one-hot indicators; ind is partition-major [dstrow, chunk*128+e] so
        # a whole superchunk loads as one [128, 1024] DMA
        ind = np.zeros((P, ES), BF16)
        indt = np.zeros((ES, P), BF16)                # [chunk*128 + e, dstrow]
        ch = np.arange(ES) // P
        e_in = np.arange(ES) % P
        real = dstrow >= 0
        r = np.nonzero(real)[0]
        ind[dstrow[r], ch[r] * P + e_in[r]] = 1
        indt[ch[r] * P + e_in[r], dstrow[r]] = 1
        # dma_gather wrapped idx layout, horizontally concatenated per
        # superchunk: [128, nsc*64]
        def wrap(ids):
            lay = np.zeros((P, nsc * IW), np.int16)
            for sc in range(nsc):
                blk = ids[sc * KP : (sc + 1) * KP]
                wr = np.zeros((16, IW), np.int16)
                kk = np.arange(KP)
                wr[kk % 16, kk // 16] = blk
                lay[:, sc * IW : (sc + 1) * IW] = np.tile(wr, (8, 1))
            return lay
        out.append(dict(gidx1=wrap(src_loc), gidx2=wrap(src_glb), ind=ind, indt=indt))
    return out, cw, nchunk, nsc, chunk_win


# ----------------------------------------------------------------------------
# program build
# ----------------------------------------------------------------------------

def build_program(nchunk, nsc, chunk_win, reps=1, single=False, upto=4, xray=0):
    import concourse.tile as tile
    from concourse import bacc, mybir
    from concourse.masks import make_identity

    NT = (N + P - 1) // P                              # 79 node tiles
    # chunk boundaries: first/last chunk of each window
    win_first = {}
    win_last = {}
    for ci, w in enumerate(chunk_win):
        if w not in win_first:
            win_first[w] = ci
        win_last[w] = ci

    nc = bacc.Bacc("TRN2", target_bir_lowering=False, debug=False,
                   num_devices=1 if single else NCORES, num_swdge_queues=4)
    dt = mybir.dt
    featP = nc.declare_dram_parameter("featP", [P, N, 2], dt.bfloat16, isOutput=False)
    W1P = nc.declare_dram_parameter("W1P", [P, 2, 264], dt.bfloat16, isOutput=False)
    W2P = nc.declare_dram_parameter("W2P", [P, 2, T2W], dt.bfloat16, isOutput=False)
    gidx1 = nc.declare_dram_parameter("gidx1", [P, nsc * IW], dt.int16, isOutput=False)
    gidx2 = nc.declare_dram_parameter("gidx2", [P, nsc * IW], dt.int16, isOutput=False)
    indp = nc.declare_dram_parameter("ind", [P, nchunk * P], dt.bfloat16, isOutput=False)
    indtp = nc.declare_dram_parameter("indt", [nchunk * P, P], dt.bfloat16, isOutput=False)
    outp = nc.declare_dram_parameter("out", [OWN, OUTF], dt.float32, isOutput=True)

    t1tabs = [nc.dram_tensor(f"t1tab{i}", [N, ROW1], dt.float8e4) for i in range(2)]
    t2owns = [nc.dram_tensor(f"t2own{i}", [T2OWN_ROWS, ROW2], dt.bfloat16) for i in range(2)]

    with tile.TileContext(nc) as tc:
        with (
            tc.tile_pool(name="const", bufs=1) as constp,
            tc.tile_pool(name="res", bufs=max(1, nchunk)) as respool,
            tc.tile_pool(name="dram", bufs=1, space="DRAM") as dramp,
        ):
            ident = constp.tile([P, P], dt.float32)
            make_identity(nc, ident[:])
            zero52 = constp.tile([P, T2W], dt.bfloat16)
            nc.vector.memset(zero52[:], 0)

            # resident constants: weights, gather indices, IndT tiles
            w1sb = constp.tile([P, 2, 264], dt.bfloat16)
            nc.sync.dma_start(out=w1sb[:], in_=W1P[:, :, :])
            w2sb = constp.tile([P, 2, T2W], dt.bfloat16)
            nc.sync.dma_start(out=w2sb[:], in_=W2P[:, :, :])
            gi1 = constp.tile([P, nsc * IW], dt.int16)
            nc.sync.dma_start(out=gi1[:], in_=gidx1[:, :])
            gi2 = constp.tile([P, nsc * IW], dt.int16)
            nc.sync.dma_start(out=gi2[:], in_=gidx2[:, :])
            indt_tiles = []
            for ci in range(nchunk):
                t = respool.tile([P, P], dt.bfloat16, tag="res")
                nc.sync.dma_start(out=t[:], in_=indtp[ci * P : (ci + 1) * P, :])
                indt_tiles.append(t)

            t2all0 = dramp.tile([N, ROW2], dt.bfloat16, tag="t2all0")
            t2all1 = dramp.tile([N, ROW2], dt.bfloat16, tag="t2all1")
            t2alls = [t2all0, t2all1]

            with (
                tc.tile_pool(name="pa", bufs=4) as pa,
                tc.tile_pool(name="pb", bufs=3) as pb,
                tc.tile_pool(name="pbw", bufs=2) as pbw,
                tc.tile_pool(name="pbfin", bufs=2) as pbfin,
                tc.tile_pool(name="pd", bufs=3) as pd,
                tc.tile_pool(name="pdw", bufs=2) as pdw,
                tc.tile_pool(name="pdfin", bufs=2) as pdfin,
                tc.tile_pool(name="ps8", bufs=1, space="PSUM") as ps8,
            ):
              for r in range(reps):
                last = r == reps - 1
                t1tab = t1tabs[r % 2]
                t2own = t2owns[r % 2]
                t2all = t2alls[r % 2]
                # ---------------- phase A: T1 table ----------------
                if True:
                    # 256 nodes (2 tiles) per iteration: 1 load, 1 write
                    NB = N // (2 * P)                  # 39 full pairs
                    for nb in range(NB):
                        lt = pa.tile([P, 2 * P, 2], dt.bfloat16, tag="lt")
                        nc.sync.dma_start(out=lt[:], in_=featP[:, nb * 2 * P : (nb + 1) * 2 * P, :])
                        row = pa.tile([P, 2, 272], dt.float8e4, tag="row")
                        for a in range(2):
                            ps = ps8.tile([P, 264], dt.float32, space="PSUM", tag="paps", bufs=2)
                            sl = slice(a * P, (a + 1) * P)
                            nc.tensor.matmul(ps[:], lhsT=lt[:, sl, 0], rhs=w1sb[:, 0, :], start=True, stop=False)
                            nc.tensor.matmul(ps[:], lhsT=lt[:, sl, 1], rhs=w1sb[:, 1, :], start=False, stop=True)
                            nc.vector.tensor_copy(row[:, a, 0:256], ps[:, 0:256])
                            nc.vector.tensor_copy(row[:, a, 256:272].bitcast(dt.bfloat16), ps[:, 256:264])
                        nc.scalar.dma_start(
                            out=t1tab[nb * 2 * P : (nb + 1) * 2 * P, 0:272].rearrange("(a p) c -> p a c", a=2),
                            in_=row[:],
                        )
                    # tail: nodes 9984:10000
                    mt = N - NB * 2 * P
                    ltt = pa.tile([P, mt, 2], dt.bfloat16, tag="ltt")
                    nc.sync.dma_start(out=ltt[:], in_=featP[:, NB * 2 * P : N, :])
                    ps = ps8.tile([P, 264], dt.float32, space="PSUM", tag="paps", bufs=2)
                    nc.tensor.matmul(ps[0:mt, :], lhsT=ltt[:, :, 0], rhs=w1sb[:, 0, :], start=True, stop=False)
                    nc.tensor.matmul(ps[0:mt, :], lhsT=ltt[:, :, 1], rhs=w1sb[:, 1, :], start=False, stop=True)
                    rowt = pa.tile([P, 272], dt.float8e4, tag="rowt")
                    nc.vector.tensor_copy(rowt[0:mt, 0:256], ps[0:mt, 0:256])
                    nc.vector.tensor_copy(rowt[0:mt, 256:272].bitcast(dt.bfloat16), ps[0:mt, 256:264])
                    nc.scalar.dma_start(out=t1tab[NB * 2 * P : N, 0:272], in_=rowt[0:mt, :])

                if upto < 2:
                    continue
                # zero t2own pad rows once
                nc.gpsimd.dma_start(out=t2own[OWN:T2OWN_ROWS, 0:T2W], in_=zero52[0 : T2OWN_ROWS - OWN, :])

                # ---------------- phase B: layer-1 edge phase ----------------
                if True:
                    # all windows' er vectors in one strided DMA
                    er_all = pbw.tile([P, NWIN, 4], dt.bfloat16, tag="erall")
                    nc.sync.dma_start(
                        out=er_all[:],
                        in_=t1tab[0 : NWIN * P, 264:272].bitcast(dt.bfloat16).rearrange("(w p) c -> p w c", p=P),
                    )
                    bst = {}
                    bwin = {"psum": None}
                    erpBs = {}

                    def b_stage0(sc):
                        if sc % 8 == 0:
                            erpBs[sc // 8] = ps8.tile([P, 512], dt.float32, space="PSUM", tag="erpB", bufs=1, name="erpB")
                        g = pb.tile([P, K, ROW1], dt.float8e4, tag="g", name="g")
                        msg = pb.tile([P, K, 260], dt.bfloat16, tag="msg", name="msg")
                        for q in range(KP // 1024):
                            nc.gpsimd.dma_gather(
                                g[:, 8 * q : 8 * (q + 1), :], t1tab[:, :],
                                gi1[:, sc * IW + 64 * q : sc * IW + 64 * (q + 1)],
                                1024, 1024, ROW1, queue_num=0 if single else (2 * sc + q) % 4)
                        ind_b = pb.tile([P, KP], dt.bfloat16, tag="ind", name="ind_b")
                        nc.scalar.dma_start(out=ind_b[:], in_=indp[:, sc * KP : (sc + 1) * KP])
                        bst[sc] = (g, msg, ind_b)

                    def b_stage1(sc):
                        g, msg, ind_b = bst[sc]
                        er_psum = erpBs[sc // 8][:, (sc % 8) * 64 : (sc % 8) * 64 + K * 4]
                        for j in range(K):
                            w = chunk_win[sc * K + j]
                            nc.tensor.matmul(
                                er_psum[:, j * 4 : (j + 1) * 4],
                                lhsT=ind_b[:, j * P : (j + 1) * P], rhs=er_all[:, w, :],
                                start=True, stop=True,
                            )
                        att = pb.tile([P, K, 4], dt.float32, tag="att", name="att")
                        nc.vector.tensor_tensor(
                            out=att[:], in0=g[:, :, 256:264].bitcast(dt.bfloat16),
                            in1=er_psum[:].rearrange("p (c h) -> p c h", c=K),
                            op=mybir.AluOpType.add,
                        )
                        att2 = pb.tile([P, K, 4], dt.float32, tag="att2", name="att2")
                        nc.vector.tensor_scalar_mul(att2[:], att[:], NEG)
                        nc.vector.tensor_tensor(out=att[:], in0=att[:], in1=att2[:], op=mybir.AluOpType.max)
                        nc.scalar.activation(msg[:, :, 256:260], att[:], mybir.ActivationFunctionType.Exp)
                        nc.vector.tensor_tensor(
                            out=msg[:, :, 0:HD].rearrange("p c (h d) -> p c h d", h=H),
                            in0=g[:, :, 0:HD].rearrange("p c (h d) -> p c h d", h=H),
                            in1=msg[:, :, 256:260, None].broadcast_to([P, K, 4, D]),
                            op=mybir.AluOpType.mult,
                        )

                    def b_stage2(sc):
                        g, msg, ind_b = bst.pop(sc)
                        for j in range(K):
                            ci = sc * K + j
                            w = chunk_win[ci]
                            if ci == win_first[w]:
                                bwin["psum"] = ps8.tile([P, 260], dt.float32, space="PSUM", tag="accB", bufs=2, name="accB")
                            win_psum = bwin["psum"]
                            nc.tensor.matmul(
                                win_psum[:],
                                lhsT=indt_tiles[ci][:],
                                rhs=msg[:, j, 0:260],
                                start=(ci == win_first[w]),
                                stop=(ci == win_last[w]),
                            )
                            if ci == win_last[w]:
                                m = WIN_SIZES[w]
                                den = pbfin.tile([P, 4], dt.float32, tag="den", name="den")
                                nc.vector.tensor_scalar_max(den[:], win_psum[:, 256:260], 1e-9)
                                rec = pbfin.tile([P, 4], dt.float32, tag="rec", name="rec")
                                nc.vector.reciprocal(rec[:], den[:])
                                h_sb = pbfin.tile([P, HD], dt.float32, tag="hsb", name="h_sb")
                                nc.vector.tensor_tensor(
                                    out=h_sb[:].rearrange("p (h d) -> p h d", h=H),
                                    in0=win_psum[:, 0:HD].rearrange("p (h d) -> p h d", h=H),
                                    in1=rec[:, :, None].broadcast_to([P, H, D]),
                                    op=mybir.AluOpType.mult,
                                )
                                hneg = pbfin.tile([P, HD], dt.float32, tag="hneg", name="hneg")
                                nc.vector.tensor_scalar_min(hneg[:], h_sb[:], 0.0)
                                hexp = pbfin.tile([P, HD], dt.float32, tag="hexp", name="hexp")
                                nc.scalar.activation(hexp[:], hneg[:], mybir.ActivationFunctionType.Exp)
                                nc.vector.tensor_scalar_max(h_sb[:], h_sb[:], 0.0)
                                nc.vector.tensor_tensor(out=h_sb[:], in0=h_sb[:], in1=hexp[:], op=mybir.AluOpType.add)
                                nc.vector.tensor_scalar_add(h_sb[:], h_sb[:], -1.0)
                                hT = pbfin.tile([P, 2, P], dt.bfloat16, tag="hT", name="hT")
                                for half in range(2):
                                    tp = ps8.tile([P, P], dt.float32, space="PSUM", tag="fin", bufs=1, name="tp")
                                    nc.tensor.transpose(out=tp[:, 0:m], in_=h_sb[0:m, half * P : (half + 1) * P], identity=ident[0:m, 0:m])
                                    nc.vector.tensor_copy(hT[:, half, 0:m], tp[:, 0:m])
                                t2ps_full = ps8.tile([P, P], dt.float32, space="PSUM", tag="fin", bufs=1, name="t2ps")
                                t2ps = t2ps_full[:, 0:T2W]
                                nc.tensor.matmul(t2ps[0:m, :], lhsT=hT[:, 0, 0:m], rhs=w2sb[:, 0, :], start=True, stop=False)
                                nc.tensor.matmul(t2ps[0:m, :], lhsT=hT[:, 1, 0:m], rhs=w2sb[:, 1, :], start=False, stop=True)
                                t2row = pbfin.tile([P, T2W], dt.bfloat16, tag="t2row", name="t2row")
                                nc.vector.tensor_copy(t2row[0:m, :], t2ps[0:m, :])
                                nc.sync.dma_start(out=t2own[w * P : w * P + m, 0:T2W], in_=t2row[0:m, :])

                    for t in range(nsc + 2):
                        if t < nsc:
                            b_stage0(t)
                        if 1 <= t < nsc + 1:
                            b_stage1(t - 1)
                        if t >= 2:
                            b_stage2(t - 2)

                if upto < 3:
                    continue
                # ---------------- phase C: allgather T2 ----------------
                t2own_bounce = dramp.tile([OWN, ROW2], dt.bfloat16, tag=f"t2b{r % 2}")
                nc.gpsimd.dma_start(out=t2own_bounce[:], in_=t2own[0:OWN, :])
                if single:
                    # analysis-only stand-in for the collective (TimelineSim
                    # cannot model collectives): keep the dataflow deps
                    nc.gpsimd.dma_start(out=t2all[0:OWN, :], in_=t2own_bounce[:])
                else:
                    nc.gpsimd.collective_compute(
                        "AllGather",
                        mybir.AluOpType.bypass,
                        replica_groups=[list(range(NCORES))],
                        ins=[t2own_bounce.opt()],
                        outs=[t2all.opt()],
                    )

                if upto < 4:
                    continue
                # ---------------- phase D: layer-2 edge phase ----------------
                if True:
                    er2_all = pdw.tile([P, NWIN, 1], dt.bfloat16, tag="er2all")
                    nc.sync.dma_start(
                        out=er2_all[:],
                        in_=t2own[0 : NWIN * P, 48:49].rearrange("(w p) c -> p w c", p=P),
                    )
                    erpD = ps8.tile([P, 512], dt.float32, space="PSUM", tag="erpD", bufs=1)
                    accD = ps8.tile([P, 512], dt.float32, space="PSUM", tag="accD", bufs=1)
                    dst_tiles = {}
                    dwin = {"psum": None}

                    def d_stage0(sc):
                        g2 = pd.tile([P, K, ROW2], dt.bfloat16, tag="g2", name="g2")
                        for q in range(KP // 1024):
                            nc.gpsimd.dma_gather(
                                g2[:, 8 * q : 8 * (q + 1), :], t2all[:, :],
                                gi2[:, sc * IW + 64 * q : sc * IW + 64 * (q + 1)],
                                1024, 1024, ROW2, queue_num=0 if single else (2 * sc + q) % 4)
                        ind_b = pd.tile([P, KP], dt.bfloat16, tag="ind2", name="ind_b2")
                        nc.gpsimd.dma_start(out=ind_b[:], in_=indp[:, sc * KP : (sc + 1) * KP])
                        dst_tiles[sc] = (g2, ind_b)

                    def d_stage1(sc):
                        g2, ind_b = dst_tiles[sc]
                        er_psum2 = erpD[:, sc * K : (sc + 1) * K]
                        for j in range(K):
                            w = chunk_win[sc * K + j]
                            nc.tensor.matmul(
                                er_psum2[:, j : j + 1],
                                lhsT=ind_b[:, j * P : (j + 1) * P], rhs=er2_all[:, w, :],
                                start=True, stop=True,
                            )
                        att = pd.tile([P, K], dt.float32, tag="attl2", name="att")
                        nc.vector.tensor_tensor(
                            out=att[:, :, None], in0=g2[:, :, 47:48], in1=er_psum2[:, :, None],
                            op=mybir.AluOpType.add,
                        )
                        att2 = pd.tile([P, K], dt.float32, tag="attl2b", name="att2")
                        nc.vector.tensor_scalar_mul(att2[:], att[:], NEG)
                        nc.vector.tensor_tensor(out=att[:], in0=att[:], in1=att2[:], op=mybir.AluOpType.max)
                        nc.scalar.activation(g2[:, :, 49:50], att[:, :, None], mybir.ActivationFunctionType.Exp)
                        nc.vector.tensor_tensor(
                            out=g2[:, :, 0:48],
                            in0=g2[:, :, 0:48],
                            in1=g2[:, :, 49:50].broadcast_to([P, K, 48]),
                            op=mybir.AluOpType.mult,
                        )

                    def d_stage2(sc, last):
                        g2, ind_b = dst_tiles.pop(sc)
                        for j in range(K):
                            ci = sc * K + j
                            w = chunk_win[ci]
                            if ci == win_first[w]:
                                dwin["psum"] = accD[:, (w % 4) * P : (w % 4) * P + 50]
                            win_psum2 = dwin["psum"]
                            nc.tensor.matmul(
                                win_psum2[:],
                                lhsT=indt_tiles[ci][:],
                                rhs=g2[:, j, 0:50],
                                start=(ci == win_first[w]),
                                stop=(ci == win_last[w]),
                            )
                            if ci == win_last[w]:
                                m = WIN_SIZES[w]
                                den = pdfin.tile([P, 1], dt.float32, tag="den2", name="den")
                                nc.vector.tensor_scalar_max(den[:], win_psum2[:, 49:50], 1e-9)
                                rec = pdfin.tile([P, 1], dt.float32, tag="rec2", name="rec")
                                nc.vector.reciprocal(rec[:], den[:])
                                logit = pdfin.tile([P, OUTF], dt.float32, tag="logit", name="logit")
                                nc.vector.tensor_scalar(
                                    out=logit[:], in0=win_psum2[:, 0:OUTF],
                                    scalar1=rec[:, 0:1], scalar2=None,
                                    op0=mybir.AluOpType.mult,
                                )
                                mx = pdfin.tile([P, 1], dt.float32, tag="mx", name="mx")
                                nc.vector.tensor_reduce(mx[:], logit[:], mybir.AxisListType.X, mybir.AluOpType.max)
                                nc.vector.tensor_scalar(
                                    out=logit[:], in0=logit[:], scalar1=mx[:, 0:1], scalar2=None,
                                    op0=mybir.AluOpType.subtract,
                                )
                                exps = pdfin.tile([P, OUTF], dt.float32, tag="exps", name="exps")
                                se = pdfin.tile([P, 1], dt.float32, tag="se", name="se")
                                nc.scalar.activation(exps[:], logit[:], mybir.ActivationFunctionType.Exp, accum_out=se[:])
                                lse = pdfin.tile([P, 1], dt.float32, tag="lse", name="lse")
                                nc.scalar.activation(lse[:], se[:], mybir.ActivationFunctionType.Ln)
                                nc.vector.tensor_scalar(
                                    out=logit[:], in0=logit[:], scalar1=lse[:, 0:1], scalar2=None,
                                    op0=mybir.AluOpType.subtract,
                                )
                                if last:
                                    nc.sync.dma_start(out=outp[w * P : w * P + m, :], in_=logit[0:m, :])

                    for t in range(nsc + 2):
                        if t < nsc:
                            d_stage0(t)
                        if 1 <= t < nsc + 1:
                            d_stage1(t - 1)
                        if t >= 2:
                            d_stage2(t - 2, last)
    nc.compile()
    return nc


# ----------------------------------------------------------------------------
# host entry
# ----------------------------------------------------------------------------

def _host_inputs(features, src, dst, W1, al1, ar1, W2, al2, ar2):
    feats = np.asarray(features, np.float32)
    W1 = np.asarray(W1, np.float32)
    W2 = np.asarray(W2, np.float32)
    al1 = np.asarray(al1, np.float32)
    ar1 = np.asarray(ar1, np.float32)
    al2 = np.asarray(al2, np.float32)
    ar2 = np.asarray(ar2, np.float32)

    Wl1 = np.stack([W1[:, h * D : (h + 1) * D] @ al1[h] for h in range(H)], axis=1)
    Wr1 = np.stack([W1[:, h * D : (h + 1) * D] @ ar1[h] for h in range(H)], axis=1)
    W1p = np.concatenate([W1, Wl1, Wr1], axis=1).astype(BF16)          # [256, 264]
    Wl2 = (W2 @ al2[0])[:, None]
    Wr2 = (W2 @ ar2[0])[:, None]
    W2p = np.concatenate([W2, Wl2, Wr2, np.zeros((HD, 3), np.float32)], axis=1).astype(BF16)  # [256, 52]
    W1P = np.ascontiguousarray(W1p.reshape(2, P, 264).transpose(1, 0, 2))
    W2P = np.ascontiguousarray(W2p.reshape(2, P, T2W).transpose(1, 0, 2))

    graph, cw, nchunk, nsc, chunk_win = _prep_graph(src, dst)
    featT = np.ascontiguousarray(feats.T)                               # [256, N]
    in_maps = []
    for c in range(NCORES):
        featTl = np.roll(featT, -OWN * c, axis=1)                       # local node order
        featPc = np.ascontiguousarray(
            featTl.reshape(2, P, N).transpose(1, 2, 0)).astype(BF16)    # [128, N, 2]
        in_maps.append(dict(
            featP=featPc,
            W1P=W1P, W2P=W2P,
            gidx1=graph[c]["gidx1"], gidx2=graph[c]["gidx2"],
            ind=graph[c]["ind"], indt=graph[c]["indt"],
        ))
    return in_maps, nchunk, nsc, chunk_win


def kernel(features, src, dst, W1, al1, ar1, W2, al2, ar2):
    from concourse.bass_utils import run_bass_kernel_spmd

    in_maps, nchunk, nsc, chunk_win = _host_inputs(
        features, src, dst, W1, al1, ar1, W2, al2, ar2)
    key = (nchunk, nsc, tuple(chunk_win))
    if key not in _CACHE:
        _CACHE[key] = build_program(nchunk, nsc, chunk_win, reps=1)
    nc = _CACHE[key]
    res = run_bass_kernel_spmd(nc, in_maps, core_ids=list(range(NCORES)))
    return np.concatenate([res.results[c]["out"] for c in range(NCORES)], axis=0)


# revision 32
# speedup vs baseline: 1.0336x; 1.0336x over previous
"""Trainium2 Bass kernel for 2-layer GAT (nn_GAT_34832184770812).

Strategy (8 NeuronCores, dst-node sharded):
- Each core owns 1250 dst nodes; node ids are rotated per core so own nodes
  are local rows 0:1250 (keeps the SPMD program identical across cores).
- Phase A: T1 = features @ [W1 | W1@al1 | W1@ar1] (bf16, replicated) ->
  DRAM gather table T1tab[N, 384] (feat 256 | el 256:260 | er 260:264 | pad).
- Phase B (layer-1 edge phase): edges sorted by dst window (128 dst rows per
  window, padded to 128-edge chunks, chunk counts uniform across cores).
  Per 1024-edge superchunk: one dma_gather of src rows (Pool/SWDGE — no
  HWDGE); one batched load of the one-hot dst indicators for all 8 chunks;
  er per edge via indicator matmuls; e = lrelu(el+er), ex = exp(e) (softmax
  is shift-invariant, exponents are O(1)); messages scaled by ex; segment
  sum over dst via indicator-transpose matmul (IndT resident in SBUF), with
  ex as extra columns producing softmax denominators in the same psum.
- Window finalize: normalize, ELU, transpose (PE), T2own = h @ W2p.
- AllGather T2own (bf16, 52-wide) -> expand to 256B rows -> T2all gather
  table.
- Phase D (layer-2 edge phase): same structure, 1 head, 47 feats.
- log_softmax per window, output [1250, 47] f32 per core, host concat.

DMA-issue budget drives the design: every HWDGE dma_start costs ~630ns on a
device shared by all queues, so host-constant tables (indices, weights,
indicator transposes) are loaded once outside the rep loop, per-chunk loads
are batched per superchunk, and gathers/bulk copies ride the Pool-engine
SWDGE path which bypasses HWDGE entirely.
"""

import numpy as np
import ml_dtypes

BF16 = ml_dtypes.bfloat16

# problem constants (hardcoded per contract)
N = 10000
E = 320000
IN_FEATS = 256
H = 4
D = 64
HD = 256
OUTF = 47
NEG = 0.2
NCORES = 8
OWN = N // NCORES          # 1250
P = 128
NWIN = (OWN + P - 1) // P  # 10 windows (last has 98 nodes)
WIN_SIZES = [min(P, OWN - P * w) for w in range(NWIN)]
K = 16                     # chunks per superchunk
KP = K * P                 # edges per superchunk
IW = KP // 16              # idx cols per superchunk (wrapped 16-row layout)
ROW1 = 512                 # T1 gather row (fp8): feat 0:256 | el bf16 @bytes 256:264 | er bf16 @bytes 264:272 | pad
ROW2 = 128                 # T2 gather row (bf16): feat 0:47 | el2 47 | er2 48 | ex2 49 | pad
T2W = 52                   # t2own row width (47 feat + el + er + ex + pad to 52)
T2OWN_ROWS = NWIN * P      # 1280 (rows 1250:1280 zeroed)

_CACHE = {}


# ----------------------------------------------------------------------------
# host-side graph preprocessing
# ----------------------------------------------------------------------------

def _prep_graph(src, dst):
    """Per-core edge partition, window sort, uniform padding, one-hots."""
    src = np.asarray(src).astype(np.int64)
    dst = np.asarray(dst).astype(np.int64)
    core_of = dst // OWN
    per_core = []
    for c in range(NCORES):
        sel = np.nonzero(core_of == c)[0]
        dl = dst[sel] - OWN * c                       # local dst in [0, OWN)
        sl = (src[sel] - OWN * c) % N                 # local src
        # sort by (window, src) — src-ascending gathers get HBM locality;
        # dstrow within a chunk is free (one-hots encode it)
        order = np.lexsort((sl, dl // P))
        per_core.append((dl[order], sl[order], src[sel][order]))

    # uniform chunks per window across cores
    cw = []
    bounds = []
    for c in range(NCORES):
        dl = per_core[c][0]
        b = np.searchsorted(dl, [P * w for w in range(NWIN + 1)])
        bounds.append(b)
    for w in range(NWIN):
        mx = max(bounds[c][w + 1] - bounds[c][w] for c in range(NCORES))
        cw.append((int(mx) + P - 1) // P)
    nchunk = sum(cw)
    nsc = (nchunk + K - 1) // K
    pad_chunks = nsc * K - nchunk
    cw[-1] += pad_chunks
    nchunk = nsc * K

    chunk_win = []
    for w in range(NWIN):
        chunk_win += [w] * cw[w]

    ES = nchunk * P
    out = []
    for c in range(NCORES):
        dl, sl, sg = per_core[c]
        b = bounds[c]
        src_loc = np.zeros(ES, np.int16)
        src_glb = np.zeros(ES, np.int16)
        dstrow = np.full(ES, -1, np.int32)            # -1 = dummy
        pos = 0
        for w in range(NWIN):
            e0, e1 = b[w], b[w + 1]
            n = e1 - e0
            src_loc[pos : pos + n] = sl[e0:e1]
            src_glb[pos : pos + n] = sg[e0:e1]
            dstrow[pos : pos + n] = dl[e0:e1] - P * w
            pos += cw[w] * P
        # one-hot indicators; ind is partition-major [dstrow, chunk*128+e] so
        # a whole superchunk loads as one [128, 1024] DMA
        ind = np.zeros((P, ES), BF16)
        indt = np.zeros((ES, P), BF16)                # [chunk*128 + e, dstrow]
        ch = np.arange(ES) // P
        e_in = np.arange(ES) % P
        real = dstrow >= 0
        r = np.nonzero(real)[0]
        ind[dstrow[r], ch[r] * P + e_in[r]] = 1
        indt[ch[r] * P + e_in[r], dstrow[r]] = 1
        # dma_gather wrapped idx layout, horizontally concatenated per
        # superchunk: [128, nsc*64]
        def wrap(ids):
            lay = np.zeros((P, nsc * IW), np.int16)
            for sc in range(nsc):
                blk = ids[sc * KP : (sc + 1) * KP]
                wr = np.zeros((16, IW), np.int16)
                kk = np.arange(KP)
                wr[kk % 16, kk // 16] = blk
                lay[:, sc * IW : (sc + 1) * IW] = np.tile(wr, (8, 1))
            return lay
        out.append(dict(gidx1=wrap(src_loc), gidx2=wrap(src_glb), ind=ind, indt=indt))
    return out, cw, nchunk, nsc, chunk_win


# ----------------------------------------------------------------------------
# program build
# ----------------------------------------------------------------------------

def build_program(nchunk, nsc, chunk_win, reps=1, single=False, upto=4, xray=0):
    import concourse.tile as tile
    from concourse import bacc, mybir
    from concourse.masks import make_identity

    NT = (N + P - 1) // P                              # 79 node tiles
    # chunk boundaries: first/last chunk of each window
    win_first = {}
    win_last = {}
    for ci, w in enumerate(chunk_win):
        if w not in win_first:
            win_first[w] = ci
        win_last[w] = ci

    nc = bacc.Bacc("TRN2", target_bir_lowering=False, debug=False,
                   num_devices=1 if single else NCORES, num_swdge_queues=4)
    dt = mybir.dt
    featP = nc.declare_dram_parameter("featP", [P, N, 2], dt.bfloat16, isOutput=False)
    W1P = nc.declare_dram_parameter("W1P", [P, 2, 264], dt.bfloat16, isOutput=False)
    W2P = nc.declare_dram_parameter("W2P", [P, 2, T2W], dt.bfloat16, isOutput=False)
    gidx1 = nc.declare_dram_parameter("gidx1", [P, nsc * IW], dt.int16, isOutput=False)
    gidx2 = nc.declare_dram_parameter("gidx2", [P, nsc * IW], dt.int16, isOutput=False)
    indp = nc.declare_dram_parameter("ind", [P, nchunk * P], dt.bfloat16, isOutput=False)
    indtp = nc.declare_dram_parameter("indt", [nchunk * P, P], dt.bfloat16, isOutput=False)
    outp = nc.declare_dram_parameter("out", [OWN, OUTF], dt.float32, isOutput=True)

    t1tabs = [nc.dram_tensor(f"t1tab{i}", [N, ROW1], dt.float8e4) for i in range(2)]
    t2owns = [nc.dram_tensor(f"t2own{i}", [T2OWN_ROWS, ROW2], dt.bfloat16) for i in range(2)]

    with tile.TileContext(nc) as tc:
        with (
            tc.tile_pool(name="const", bufs=1) as constp,
            tc.tile_pool(name="res", bufs=max(1, nchunk)) as respool,
            tc.tile_pool(name="dram", bufs=1, space="DRAM") as dramp,
        ):
            ident = constp.tile([P, P], dt.float32)
            make_identity(nc, ident[:])
            zero52 = constp.tile([P, T2W], dt.bfloat16)
            nc.vector.memset(zero52[:], 0)

            # resident constants: weights, gather indices, IndT tiles
            w1sb = constp.tile([P, 2, 264], dt.bfloat16)
            nc.sync.dma_start(out=w1sb[:], in_=W1P[:, :, :])
            w2sb = constp.tile([P, 2, T2W], dt.bfloat16)
            nc.sync.dma_start(out=w2sb[:], in_=W2P[:, :, :])
            gi1 = constp.tile([P, nsc * IW], dt.int16)
            nc.sync.dma_start(out=gi1[:], in_=gidx1[:, :])
            gi2 = constp.tile([P, nsc * IW], dt.int16)
            nc.sync.dma_start(out=gi2[:], in_=gidx2[:, :])
            indt_tiles = []
            for ci in range(nchunk):
                t = respool.tile([P, P], dt.bfloat16, tag="res")
                nc.sync.dma_start(out=t[:], in_=indtp[ci * P : (ci + 1) * P, :])
                indt_tiles.append(t)

            t2all0 = dramp.tile([N, ROW2], dt.bfloat16, tag="t2all0")
            t2all1 = dramp.tile([N, ROW2], dt.bfloat16, tag="t2all1")
            t2alls = [t2all0, t2all1]

            with (
                tc.tile_pool(name="pa", bufs=4) as pa,
                tc.tile_pool(name="pb", bufs=3) as pb,
                tc.tile_pool(name="pbw", bufs=2) as pbw,
                tc.tile_pool(name="pbfin", bufs=2) as pbfin,
                tc.tile_pool(name="pd", bufs=3) as pd,
                tc.tile_pool(name="pdw", bufs=2) as pdw,
                tc.tile_pool(name="pdfin", bufs=2) as pdfin,
                tc.tile_pool(name="ps8", bufs=1, space="PSUM") as ps8,
            ):
              def a_closures(rr):
                """Phase A for rep rr as per-iteration closures (interleavable)."""
                t1 = t1tabs[rr % 2]
                NB = N // (2 * P)                      # 39 full pairs + tail
                fns = []

                def full_iter(nb):
                    def f():
                        lt = pa.tile([P, 2 * P, 2], dt.bfloat16, tag="lt", name="lt")
                        nc.sync.dma_start(out=lt[:], in_=featP[:, nb * 2 * P : (nb + 1) * 2 * P, :])
                        row = pa.tile([P, 2, 272], dt.float8e4, tag="row", name="row")
                        for a in range(2):
                            ps = ps8.tile([P, 264], dt.float32, space="PSUM", tag="paps", bufs=2, name="ps")
                            sl = slice(a * P, (a + 1) * P)
                            nc.tensor.matmul(ps[:], lhsT=lt[:, sl, 0], rhs=w1sb[:, 0, :], start=True, stop=False)
                            nc.tensor.matmul(ps[:], lhsT=lt[:, sl, 1], rhs=w1sb[:, 1, :], start=False, stop=True)
                            nc.vector.tensor_copy(row[:, a, 0:256], ps[:, 0:256])
                            nc.vector.tensor_copy(row[:, a, 256:272].bitcast(dt.bfloat16), ps[:, 256:264])
                        nc.scalar.dma_start(
                            out=t1[nb * 2 * P : (nb + 1) * 2 * P, 0:272].rearrange("(a p) c -> p a c", a=2),
                            in_=row[:],
                        )
                    return f

                def tail_iter():
                    mt = N - NB * 2 * P
                    ltt = pa.tile([P, mt, 2], dt.bfloat16, tag="ltt", name="ltt")
                    nc.sync.dma_start(out=ltt[:], in_=featP[:, NB * 2 * P : N, :])
                    ps = ps8.tile([P, 264], dt.float32, space="PSUM", tag="paps", bufs=2, name="ps")
                    nc.tensor.matmul(ps[0:mt, :], lhsT=ltt[:, :, 0], rhs=w1sb[:, 0, :], start=True, stop=False)
                    nc.tensor.matmul(ps[0:mt, :], lhsT=ltt[:, :, 1], rhs=w1sb[:, 1, :], start=False, stop=True)
                    rowt = pa.tile([P, 272], dt.float8e4, tag="rowt", name="rowt")
                    nc.vector.tensor_copy(rowt[0:mt, 0:256], ps[0:mt, 0:256])
                    nc.vector.tensor_copy(rowt[0:mt, 256:272].bitcast(dt.bfloat16), ps[0:mt, 256:264])
                    nc.scalar.dma_start(out=t1[NB * 2 * P : N, 0:272], in_=rowt[0:mt, :])

                for nb in range(NB):
                    fns.append(full_iter(nb))
                fns.append(tail_iter)
                return fns

              for r in range(reps):
                last = r == reps - 1
                t1tab = t1tabs[r % 2]
                t2own = t2owns[r % 2]
                t2all = t2alls[r % 2]
                # ---------------- phase A ----------------
                # rep 0 runs inline; A(r+1) is emitted interleaved into D(r)
                if r == 0:
                    for f in a_closures(0):
                        f()

                if upto < 2:
                    continue
                # zero t2own pad rows once
                nc.gpsimd.dma_start(out=t2own[OWN:T2OWN_ROWS, 0:T2W], in_=zero52[0 : T2OWN_ROWS - OWN, :])

                # ---------------- phase B: layer-1 edge phase ----------------
                if True:
                    # all windows' er vectors in one strided DMA
                    er_all = pbw.tile([P, NWIN, 4], dt.bfloat16, tag="erall")
                    nc.sync.dma_start(
                        out=er_all[:],
                        in_=t1tab[0 : NWIN * P, 264:272].bitcast(dt.bfloat16).rearrange("(w p) c -> p w c", p=P),
                    )
                    bst = {}
                    bwin = {"psum": None}
                    erpBs = {}

                    def b_stage0(sc):
                        if sc % 8 == 0:
                            erpBs[sc // 8] = ps8.tile([P, 512], dt.float32, space="PSUM", tag="erpB", bufs=1, name="erpB")
                        g = pb.tile([P, K, ROW1], dt.float8e4, tag="g", name="g")
                        msg = pb.tile([P, K, 260], dt.bfloat16, tag="msg", name="msg")
                        for q in range(KP // 1024):
                            nc.gpsimd.dma_gather(
                                g[:, 8 * q : 8 * (q + 1), :], t1tab[:, :],
                                gi1[:, sc * IW + 64 * q : sc * IW + 64 * (q + 1)],
                                1024, 1024, ROW1, queue_num=0 if single else (2 * sc + q) % 4)
                        ind_b = pb.tile([P, KP], dt.bfloat16, tag="ind", name="ind_b")
                        nc.scalar.dma_start(out=ind_b[:], in_=indp[:, sc * KP : (sc + 1) * KP])
                        bst[sc] = (g, msg, ind_b)

                    def b_stage1(sc):
                        g, msg, ind_b = bst[sc]
                        er_psum = erpBs[sc // 8][:, (sc % 8) * 64 : (sc % 8) * 64 + K * 4]
                        for j in range(K):
                            w = chunk_win[sc * K + j]
                            nc.tensor.matmul(
                                er_psum[:, j * 4 : (j + 1) * 4],
                                lhsT=ind_b[:, j * P : (j + 1) * P], rhs=er_all[:, w, :],
                                start=True, stop=True,
                            )
                        att = pb.tile([P, K, 4], dt.float32, tag="att", name="att")
                        nc.vector.tensor_tensor(
                            out=att[:], in0=g[:, :, 256:264].bitcast(dt.bfloat16),
                            in1=er_psum[:].rearrange("p (c h) -> p c h", c=K),
                            op=mybir.AluOpType.add,
                        )
                        att2 = pb.tile([P, K, 4], dt.float32, tag="att2", name="att2")
                        nc.vector.tensor_scalar_mul(att2[:], att[:], NEG)
                        nc.vector.tensor_tensor(out=att[:], in0=att[:], in1=att2[:], op=mybir.AluOpType.max)
                        nc.scalar.activation(msg[:, :, 256:260], att[:], mybir.ActivationFunctionType.Exp)
                        nc.vector.tensor_tensor(
                            out=msg[:, :, 0:HD].rearrange("p c (h d) -> p c h d", h=H),
                            in0=g[:, :, 0:HD].rearrange("p c (h d) -> p c h d", h=H),
                            in1=msg[:, :, 256:260, None].broadcast_to([P, K, 4, D]),
                            op=mybir.AluOpType.mult,
                        )

                    def b_stage2(sc):
                        g, msg, ind_b = bst.pop(sc)
                        for j in range(K):
                            ci = sc * K + j
                            w = chunk_win[ci]
                            if ci == win_first[w]:
                                bwin["psum"] = ps8.tile([P, 260], dt.float32, space="PSUM", tag="accB", bufs=2, name="accB")
                            win_psum = bwin["psum"]
                            nc.tensor.matmul(
                                win_psum[:],
                                lhsT=indt_tiles[ci][:],
                                rhs=msg[:, j, 0:260],
                                start=(ci == win_first[w]),
                                stop=(ci == win_last[w]),
                            )
                            if ci == win_last[w]:
                                m = WIN_SIZES[w]
                                den = pbfin.tile([P, 4], dt.float32, tag="den", name="den")
                                nc.vector.tensor_scalar_max(den[:], win_psum[:, 256:260], 1e-9)
                                rec = pbfin.tile([P, 4], dt.float32, tag="rec", name="rec")
                                nc.vector.reciprocal(rec[:], den[:])
                                h_sb = pbfin.tile([P, HD], dt.float32, tag="hsb", name="h_sb")
                                nc.vector.tensor_tensor(
                                    out=h_sb[:].rearrange("p (h d) -> p h d", h=H),
                                    in0=win_psum[:, 0:HD].rearrange("p (h d) -> p h d", h=H),
                                    in1=rec[:, :, None].broadcast_to([P, H, D]),
                                    op=mybir.AluOpType.mult,
                                )
                                hneg = pbfin.tile([P, HD], dt.float32, tag="hneg", name="hneg")
                                nc.vector.tensor_scalar_min(hneg[:], h_sb[:], 0.0)
                                hexp = pbfin.tile([P, HD], dt.float32, tag="hexp", name="hexp")
                                nc.scalar.activation(hexp[:], hneg[:], mybir.ActivationFunctionType.Exp)
                                nc.vector.tensor_scalar_max(h_sb[:], h_sb[:], 0.0)
                                nc.vector.tensor_tensor(out=h_sb[:], in0=h_sb[:], in1=hexp[:], op=mybir.AluOpType.add)
                                nc.vector.tensor_scalar_add(h_sb[:], h_sb[:], -1.0)
                                hT = pbfin.tile([P, 2, P], dt.bfloat16, tag="hT", name="hT")
                                for half in range(2):
                                    tp = ps8.tile([P, P], dt.float32, space="PSUM", tag="fin", bufs=1, name="tp")
                                    nc.tensor.transpose(out=tp[:, 0:m], in_=h_sb[0:m, half * P : (half + 1) * P], identity=ident[0:m, 0:m])
                                    nc.vector.tensor_copy(hT[:, half, 0:m], tp[:, 0:m])
                                t2ps_full = ps8.tile([P, P], dt.float32, space="PSUM", tag="fin", bufs=1, name="t2ps")
                                t2ps = t2ps_full[:, 0:T2W]
                                nc.tensor.matmul(t2ps[0:m, :], lhsT=hT[:, 0, 0:m], rhs=w2sb[:, 0, :], start=True, stop=False)
                                nc.tensor.matmul(t2ps[0:m, :], lhsT=hT[:, 1, 0:m], rhs=w2sb[:, 1, :], start=False, stop=True)
                                t2row = pbfin.tile([P, T2W], dt.bfloat16, tag="t2row", name="t2row")
                                nc.vector.tensor_copy(t2row[0:m, :], t2ps[0:m, :])
                                nc.sync.dma_start(out=t2own[w * P : w * P + m, 0:T2W], in_=t2row[0:m, :])

                    for t in range(nsc + 2):
                        if t < nsc:
                            b_stage0(t)
                        if 1 <= t < nsc + 1:
                            b_stage1(t - 1)
                        if t >= 2:
                            b_stage2(t - 2)

                if upto < 3:
                    continue
                # ---------------- phase C: allgather T2 ----------------
                t2own_bounce = dramp.tile([OWN, ROW2], dt.bfloat16, tag=f"t2b{r % 2}")
                nc.gpsimd.dma_start(out=t2own_bounce[:], in_=t2own[0:OWN, :])
                if single:
                    # analysis-only stand-in for the collective (TimelineSim
                    # cannot model collectives): keep the dataflow deps
                    nc.gpsimd.dma_start(out=t2all[0:OWN, :], in_=t2own_bounce[:])
                else:
                    nc.gpsimd.collective_compute(
                        "AllGather",
                        mybir.AluOpType.bypass,
                        replica_groups=[list(range(NCORES))],
                        ins=[t2own_bounce.opt()],
                        outs=[t2all.opt()],
                    )

                if upto < 4:
                    continue
                # ---------------- phase D: layer-2 edge phase ----------------
                if True:
                    er2_all = pdw.tile([P, NWIN, 1], dt.bfloat16, tag="er2all")
                    nc.sync.dma_start(
                        out=er2_all[:],
                        in_=t2own[0 : NWIN * P, 48:49].rearrange("(w p) c -> p w c", p=P),
                    )
                    erpD = ps8.tile([P, 512], dt.float32, space="PSUM", tag="erpD", bufs=1)
                    accD = ps8.tile([P, 512], dt.float32, space="PSUM", tag="accD", bufs=1)
                    dst_tiles = {}
                    dwin = {"psum": None}

                    def d_stage0(sc):
                        g2 = pd.tile([P, K, ROW2], dt.bfloat16, tag="g2", name="g2")
                        for q in range(KP // 1024):
                            nc.gpsimd.dma_gather(
                                g2[:, 8 * q : 8 * (q + 1), :], t2all[:, :],
                                gi2[:, sc * IW + 64 * q : sc * IW + 64 * (q + 1)],
                                1024, 1024, ROW2, queue_num=0 if single else (2 * sc + q) % 4)
                        ind_b = pd.tile([P, KP], dt.bfloat16, tag="ind2", name="ind_b2")
                        nc.gpsimd.dma_start(out=ind_b[:], in_=indp[:, sc * KP : (sc + 1) * KP])
                        dst_tiles[sc] = (g2, ind_b)

                    def d_stage1(sc):
                        g2, ind_b = dst_tiles[sc]
                        er_psum2 = erpD[:, sc * K : (sc + 1) * K]
                        for j in range(K):
                            w = chunk_win[sc * K + j]
                            nc.tensor.matmul(
                                er_psum2[:, j : j + 1],
                                lhsT=ind_b[:, j * P : (j + 1) * P], rhs=er2_all[:, w, :],
                                start=True, stop=True,
                            )
                        att = pd.tile([P, K], dt.float32, tag="attl2", name="att")
                        nc.vector.tensor_tensor(
                            out=att[:, :, None], in0=g2[:, :, 47:48], in1=er_psum2[:, :, None],
                            op=mybir.AluOpType.add,
                        )
                        att2 = pd.tile([P, K], dt.float32, tag="attl2b", name="att2")
                        nc.vector.tensor_scalar_mul(att2[:], att[:], NEG)
                        nc.vector.tensor_tensor(out=att[:], in0=att[:], in1=att2[:], op=mybir.AluOpType.max)
                        nc.scalar.activation(g2[:, :, 49:50], att[:, :, None], mybir.ActivationFunctionType.Exp)
                        nc.vector.tensor_tensor(
                            out=g2[:, :, 0:48],
                            in0=g2[:, :, 0:48],
                            in1=g2[:, :, 49:50].broadcast_to([P, K, 48]),
                            op=mybir.AluOpType.mult,
                        )

                    def d_stage2(sc, last):
                        g2, ind_b = dst_tiles.pop(sc)
                        for j in range(K):
                            ci = sc * K + j
                            w = chunk_win[ci]
                            if ci == win_first[w]:
                                dwin["psum"] = accD[:, (w % 4) * P : (w % 4) * P + 50]
                            win_psum2 = dwin["psum"]
                            nc.tensor.matmul(
                                win_psum2[:],
                                lhsT=indt_tiles[ci][:],
                                rhs=g2[:, j, 0:50],
                                start=(ci == win_first[w]),
                                stop=(ci == win_last[w]),
                            )
                            if ci == win_last[w]:
                                m = WIN_SIZES[w]
                                den = pdfin.tile([P, 1], dt.float32, tag="den2", name="den")
                                nc.vector.tensor_scalar_max(den[:], win_psum2[:, 49:50], 1e-9)
                                rec = pdfin.tile([P, 1], dt.float32, tag="rec2", name="rec")
                                nc.vector.reciprocal(rec[:], den[:])
                                logit = pdfin.tile([P, OUTF], dt.float32, tag="logit", name="logit")
                                nc.vector.tensor_scalar(
                                    out=logit[:], in0=win_psum2[:, 0:OUTF],
                                    scalar1=rec[:, 0:1], scalar2=None,
                                    op0=mybir.AluOpType.mult,
                                )
                                mx = pdfin.tile([P, 1], dt.float32, tag="mx", name="mx")
                                nc.vector.tensor_reduce(mx[:], logit[:], mybir.AxisListType.X, mybir.AluOpType.max)
                                nc.vector.tensor_scalar(
                                    out=logit[:], in0=logit[:], scalar1=mx[:, 0:1], scalar2=None,
                                    op0=mybir.AluOpType.subtract,
                                )
                                exps = pdfin.tile([P, OUTF], dt.float32, tag="exps", name="exps")
                                se = pdfin.tile([P, 1], dt.float32, tag="se", name="se")
                                nc.scalar.activation(exps[:], logit[:], mybir.ActivationFunctionType.Exp, accum_out=se[:])
                                lse = pdfin.tile([P, 1], dt.float32, tag="lse", name="lse")
                                nc.scalar.activation(lse[:], se[:], mybir.ActivationFunctionType.Ln)
                                nc.vector.tensor_scalar(
                                    out=logit[:], in0=logit[:], scalar1=lse[:, 0:1], scalar2=None,
                                    op0=mybir.AluOpType.subtract,
                                )
                                if last:
                                    nc.sync.dma_start(out=outp[w * P : w * P + m, :], in_=logit[0:m, :])

                    a_next = a_closures(r + 1) if (r + 1 < reps and upto >= 2) else []
                    ai = 0
                    for t in range(nsc + 2):
                        if t < nsc:
                            d_stage0(t)
                        if 1 <= t < nsc + 1:
                            d_stage1(t - 1)
                        if t >= 2:
                            d_stage2(t - 2, last)
                        while ai * (nsc + 2) < len(a_next) * (t + 1):
                            a_next[ai]()
                            ai += 1
    nc.compile()
    return nc


# ----------------------------------------------------------------------------
# host entry
# ----------------------------------------------------------------------------

def _host_inputs(features, src, dst, W1, al1, ar1, W2, al2, ar2):
    feats = np.asarray(features, np.float32)
    W1 = np.asarray(W1, np.float32)
    W2 = np.asarray(W2, np.float32)
    al1 = np.asarray(al1, np.float32)
    ar1 = np.asarray(ar1, np.float32)
    al2 = np.asarray(al2, np.float32)
    ar2 = np.asarray(ar2, np.float32)

    Wl1 = np.stack([W1[:, h * D : (h + 1) * D] @ al1[h] for h in range(H)], axis=1)
    Wr1 = np.stack([W1[:, h * D : (h + 1) * D] @ ar1[h] for h in range(H)], axis=1)
    W1p = np.concatenate([W1, Wl1, Wr1], axis=1).astype(BF16)          # [256, 264]
    Wl2 = (W2 @ al2[0])[:, None]
    Wr2 = (W2 @ ar2[0])[:, None]
    W2p = np.concatenate([W2, Wl2, Wr2, np.zeros((HD, 3), np.float32)], axis=1).astype(BF16)  # [256, 52]
    W1P = np.ascontiguousarray(W1p.reshape(2, P, 264).transpose(1, 0, 2))
    W2P = np.ascontiguousarray(W2p.reshape(2, P, T2W).transpose(1, 0, 2))

    graph, cw, nchunk, nsc, chunk_win = _prep_graph(src, dst)
    featT = np.ascontiguousarray(feats.T)                               # [256, N]
    in_maps = []
    for c in range(NCORES):
        featTl = np.roll(featT, -OWN * c, axis=1)                       # local node order
        featPc = np.ascontiguousarray(
            featTl.reshape(2, P, N).transpose(1, 2, 0)).astype(BF16)    # [128, N, 2]
        in_maps.append(dict(
            featP=featPc,
            W1P=W1P, W2P=W2P,
            gidx1=graph[c]["gidx1"], gidx2=graph[c]["gidx2"],
            ind=graph[c]["ind"], indt=graph[c]["indt"],
        ))
    return in_maps, nchunk, nsc, chunk_win


def kernel(features, src, dst, W1, al1, ar1, W2, al2, ar2):
    from concourse.bass_utils import run_bass_kernel_spmd

    in_maps, nchunk, nsc, chunk_win = _host_inputs(
        features, src, dst, W1, al1, ar1, W2, al2, ar2)
    key = (nchunk, nsc, tuple(chunk_win))
    if key not in _CACHE:
        _CACHE[key] = build_program(nchunk, nsc, chunk_win, reps=1)
    nc = _CACHE[key]
    res = run_bass_kernel_spmd(nc, in_maps, core_ids=list(range(NCORES)))
    return np.concatenate([res.results[c]["out"] for c in range(NCORES)], axis=0)


# revision 33
# speedup vs baseline: 1.1808x; 1.1424x over previous
"""Trainium2 Bass kernel for 2-layer GAT (nn_GAT_34832184770812).

Strategy (8 NeuronCores, dst-node sharded):
- Each core owns 1250 dst nodes; node ids are rotated per core so own nodes
  are local rows 0:1250 (keeps the SPMD program identical across cores).
- Phase A: T1 = features @ [W1 | W1@al1 | W1@ar1] (bf16, replicated) ->
  DRAM gather table T1tab[N, 384] (feat 256 | el 256:260 | er 260:264 | pad).
- Phase B (layer-1 edge phase): edges sorted by dst window (128 dst rows per
  window, padded to 128-edge chunks, chunk counts uniform across cores).
  Per 1024-edge superchunk: one dma_gather of src rows (Pool/SWDGE — no
  HWDGE); one batched load of the one-hot dst indicators for all 8 chunks;
  er per edge via indicator matmuls; e = lrelu(el+er), ex = exp(e) (softmax
  is shift-invariant, exponents are O(1)); messages scaled by ex; segment
  sum over dst via indicator-transpose matmul (IndT resident in SBUF), with
  ex as extra columns producing softmax denominators in the same psum.
- Window finalize: normalize, ELU, transpose (PE), T2own = h @ W2p.
- AllGather T2own (bf16, 52-wide) -> expand to 256B rows -> T2all gather
  table.
- Phase D (layer-2 edge phase): same structure, 1 head, 47 feats.
- log_softmax per window, output [1250, 47] f32 per core, host concat.

DMA-issue budget drives the design: every HWDGE dma_start costs ~630ns on a
device shared by all queues, so host-constant tables (indices, weights,
indicator transposes) are loaded once outside the rep loop, per-chunk loads
are batched per superchunk, and gathers/bulk copies ride the Pool-engine
SWDGE path which bypasses HWDGE entirely.
"""

import numpy as np
import ml_dtypes

BF16 = ml_dtypes.bfloat16

# problem constants (hardcoded per contract)
N = 10000
E = 320000
IN_FEATS = 256
H = 4
D = 64
HD = 256
OUTF = 47
NEG = 0.2
NCORES = 8
OWN = N // NCORES          # 1250
P = 128
NWIN = (OWN + P - 1) // P  # 10 windows (last has 98 nodes)
WIN_SIZES = [min(P, OWN - P * w) for w in range(NWIN)]
K = 16                     # chunks per superchunk
KP = K * P                 # edges per superchunk
IW = KP // 16              # idx cols per superchunk (wrapped 16-row layout)
ROW1 = 512                 # T1 gather row (fp8): feat 0:256 | el bf16 @bytes 256:264 | er bf16 @bytes 264:272 | pad
ROW2 = 128                 # T2 gather row (bf16): feat 0:47 | el2 47 | er2 48 | ex2 49 | pad
T2W = 52                   # t2own row width (47 feat + el + er + ex + pad to 52)
T2OWN_ROWS = NWIN * P      # 1280 (rows 1250:1280 zeroed)

_CACHE = {}


# ----------------------------------------------------------------------------
# host-side graph preprocessing
# ----------------------------------------------------------------------------

def _prep_graph(src, dst):
    """Per-core edge partition, window sort, uniform padding, one-hots."""
    src = np.asarray(src).astype(np.int64)
    dst = np.asarray(dst).astype(np.int64)
    core_of = dst // OWN
    per_core = []
    for c in range(NCORES):
        sel = np.nonzero(core_of == c)[0]
        dl = dst[sel] - OWN * c                       # local dst in [0, OWN)
        sl = (src[sel] - OWN * c) % N                 # local src
        # sort by (window, src) — src-ascending gathers get HBM locality;
        # dstrow within a chunk is free (one-hots encode it)
        order = np.lexsort((sl, dl // P))
        per_core.append((dl[order], sl[order], src[sel][order]))

    # uniform chunks per window across cores
    cw = []
    bounds = []
    for c in range(NCORES):
        dl = per_core[c][0]
        b = np.searchsorted(dl, [P * w for w in range(NWIN + 1)])
        bounds.append(b)
    for w in range(NWIN):
        mx = max(bounds[c][w + 1] - bounds[c][w] for c in range(NCORES))
        cw.append((int(mx) + P - 1) // P)
    nchunk = sum(cw)
    nsc = (nchunk + K - 1) // K
    pad_chunks = nsc * K - nchunk
    cw[-1] += pad_chunks
    nchunk = nsc * K

    chunk_win = []
    for w in range(NWIN):
        chunk_win += [w] * cw[w]

    ES = nchunk * P
    out = []
    for c in range(NCORES):
        dl, sl, sg = per_core[c]
        b = bounds[c]
        src_loc = np.zeros(ES, np.int16)
        src_glb = np.zeros(ES, np.int16)
        dstrow = np.full(ES, -1, np.int32)            # -1 = dummy
        pos = 0
        for w in range(NWIN):
            e0, e1 = b[w], b[w + 1]
            n = e1 - e0
            src_loc[pos : pos + n] = sl[e0:e1]
            src_glb[pos : pos + n] = sg[e0:e1]
            dstrow[pos : pos + n] = dl[e0:e1] - P * w
            pos += cw[w] * P
        # one-hot indicators; ind is partition-major [dstrow, chunk*128+e] so
        # a whole superchunk loads as one [128, 1024] DMA
        ind = np.zeros((P, ES), BF16)
        indt = np.zeros((ES, P), BF16)                # [chunk*128 + e, dstrow]
        ch = np.arange(ES) // P
        e_in = np.arange(ES) % P
        real = dstrow >= 0
        r = np.nonzero(real)[0]
        ind[dstrow[r], ch[r] * P + e_in[r]] = 1
        indt[ch[r] * P + e_in[r], dstrow[r]] = 1
        # dma_gather wrapped idx layout, horizontally concatenated per
        # superchunk: [128, nsc*64]
        def wrap(ids):
            lay = np.zeros((P, nsc * IW), np.int16)
            for sc in range(nsc):
                blk = ids[sc * KP : (sc + 1) * KP]
                wr = np.zeros((16, IW), np.int16)
                kk = np.arange(KP)
                wr[kk % 16, kk // 16] = blk
                lay[:, sc * IW : (sc + 1) * IW] = np.tile(wr, (8, 1))
            return lay
        out.append(dict(gidx1=wrap(src_loc), gidx2=wrap(src_glb), ind=ind, indt=indt))
    return out, cw, nchunk, nsc, chunk_win


# ----------------------------------------------------------------------------
# program build
# ----------------------------------------------------------------------------

def build_program(nchunk, nsc, chunk_win, reps=1, single=False, upto=4, xray=0):
    import concourse.tile as tile
    from concourse import bacc, mybir
    from concourse.masks import make_identity

    NT = (N + P - 1) // P                              # 79 node tiles
    # chunk boundaries: first/last chunk of each window
    win_first = {}
    win_last = {}
    for ci, w in enumerate(chunk_win):
        if w not in win_first:
            win_first[w] = ci
        win_last[w] = ci

    nc = bacc.Bacc("TRN2", target_bir_lowering=False, debug=False,
                   num_devices=1 if single else NCORES, num_swdge_queues=4)
    dt = mybir.dt
    featP = nc.declare_dram_parameter("featP", [P, N, 2], dt.bfloat16, isOutput=False)
    W1P = nc.declare_dram_parameter("W1P", [P, 2, 264], dt.bfloat16, isOutput=False)
    W2P = nc.declare_dram_parameter("W2P", [P, 2, T2W], dt.bfloat16, isOutput=False)
    gidx1 = nc.declare_dram_parameter("gidx1", [P, nsc * IW], dt.int16, isOutput=False)
    gidx2 = nc.declare_dram_parameter("gidx2", [P, nsc * IW], dt.int16, isOutput=False)
    indp = nc.declare_dram_parameter("ind", [P, nchunk * P], dt.bfloat16, isOutput=False)
    indtp = nc.declare_dram_parameter("indt", [nchunk * P, P], dt.bfloat16, isOutput=False)
    outp = nc.declare_dram_parameter("out", [OWN, OUTF], dt.float32, isOutput=True)

    t1tabs = [nc.dram_tensor(f"t1tab{i}", [N, ROW1], dt.float8e4) for i in range(2)]
    t2owns = [nc.dram_tensor(f"t2own{i}", [T2OWN_ROWS, ROW2], dt.bfloat16) for i in range(2)]

    with tile.TileContext(nc) as tc:
        with (
            tc.tile_pool(name="const", bufs=1) as constp,
            tc.tile_pool(name="res", bufs=max(1, nchunk)) as respool,
            tc.tile_pool(name="dram", bufs=1, space="DRAM") as dramp,
        ):
            ident = constp.tile([P, P], dt.float32)
            make_identity(nc, ident[:])
            zero52 = constp.tile([P, T2W], dt.bfloat16)
            nc.vector.memset(zero52[:], 0)

            # resident constants: weights, gather indices, IndT tiles
            w1sb = constp.tile([P, 2, 264], dt.bfloat16)
            nc.sync.dma_start(out=w1sb[:], in_=W1P[:, :, :])
            w2sb = constp.tile([P, 2, T2W], dt.bfloat16)
            nc.sync.dma_start(out=w2sb[:], in_=W2P[:, :, :])
            gi1 = constp.tile([P, nsc * IW], dt.int16)
            nc.sync.dma_start(out=gi1[:], in_=gidx1[:, :])
            gi2 = constp.tile([P, nsc * IW], dt.int16)
            nc.sync.dma_start(out=gi2[:], in_=gidx2[:, :])
            indt_tiles = []
            for ci in range(nchunk):
                t = respool.tile([P, P], dt.bfloat16, tag="res")
                nc.sync.dma_start(out=t[:], in_=indtp[ci * P : (ci + 1) * P, :])
                indt_tiles.append(t)

            t2all0 = dramp.tile([N, ROW2], dt.bfloat16, tag="t2all0")
            t2all1 = dramp.tile([N, ROW2], dt.bfloat16, tag="t2all1")
            t2alls = [t2all0, t2all1]

            with (
                tc.tile_pool(name="pa", bufs=4) as pa,
                tc.tile_pool(name="pb", bufs=3) as pb,
                tc.tile_pool(name="pbw", bufs=2) as pbw,
                tc.tile_pool(name="pbfin", bufs=2) as pbfin,
                tc.tile_pool(name="pd", bufs=3) as pd,
                tc.tile_pool(name="pdw", bufs=2) as pdw,
                tc.tile_pool(name="pdfin", bufs=2) as pdfin,
                tc.tile_pool(name="ps8", bufs=1, space="PSUM") as ps8,
            ):
              def a_closures(rr):
                """Phase A for rep rr as per-iteration closures (interleavable)."""
                t1 = t1tabs[rr % 2]
                NB = N // (2 * P)                      # 39 full pairs + tail
                fns = []

                def full_iter(nb):
                    def f():
                        lt = pa.tile([P, 2 * P, 2], dt.bfloat16, tag="lt", name="lt")
                        nc.sync.dma_start(out=lt[:], in_=featP[:, nb * 2 * P : (nb + 1) * 2 * P, :])
                        row = pa.tile([P, 2, 272], dt.float8e4, tag="row", name="row")
                        for a in range(2):
                            ps = ps8.tile([P, 264], dt.float32, space="PSUM", tag="paps", bufs=2, name="ps")
                            sl = slice(a * P, (a + 1) * P)
                            nc.tensor.matmul(ps[:], lhsT=lt[:, sl, 0], rhs=w1sb[:, 0, :], start=True, stop=False)
                            nc.tensor.matmul(ps[:], lhsT=lt[:, sl, 1], rhs=w1sb[:, 1, :], start=False, stop=True)
                            nc.vector.tensor_copy(row[:, a, 0:256], ps[:, 0:256])
                            nc.vector.tensor_copy(row[:, a, 256:272].bitcast(dt.bfloat16), ps[:, 256:264])
                        nc.scalar.dma_start(
                            out=t1[nb * 2 * P : (nb + 1) * 2 * P, 0:272].rearrange("(a p) c -> p a c", a=2),
                            in_=row[:],
                        )
                    return f

                def tail_iter():
                    mt = N - NB * 2 * P
                    ltt = pa.tile([P, mt, 2], dt.bfloat16, tag="ltt", name="ltt")
                    nc.sync.dma_start(out=ltt[:], in_=featP[:, NB * 2 * P : N, :])
                    ps = ps8.tile([P, 264], dt.float32, space="PSUM", tag="paps", bufs=2, name="ps")
                    nc.tensor.matmul(ps[0:mt, :], lhsT=ltt[:, :, 0], rhs=w1sb[:, 0, :], start=True, stop=False)
                    nc.tensor.matmul(ps[0:mt, :], lhsT=ltt[:, :, 1], rhs=w1sb[:, 1, :], start=False, stop=True)
                    rowt = pa.tile([P, 272], dt.float8e4, tag="rowt", name="rowt")
                    nc.vector.tensor_copy(rowt[0:mt, 0:256], ps[0:mt, 0:256])
                    nc.vector.tensor_copy(rowt[0:mt, 256:272].bitcast(dt.bfloat16), ps[0:mt, 256:264])
                    nc.scalar.dma_start(out=t1[NB * 2 * P : N, 0:272], in_=rowt[0:mt, :])

                for nb in range(NB):
                    fns.append(full_iter(nb))
                fns.append(tail_iter)
                return fns

              for r in range(reps):
                last = r == reps - 1
                t1tab = t1tabs[r % 2]
                t2own = t2owns[r % 2]
                t2all = t2alls[r % 2]
                # ---------------- phase A ----------------
                # rep 0 runs inline; A(r+1) is emitted interleaved into D(r)
                if r == 0:
                    for f in a_closures(0):
                        f()

                if upto < 2:
                    continue
                # zero t2own pad rows once
                nc.gpsimd.dma_start(out=t2own[OWN:T2OWN_ROWS, 0:T2W], in_=zero52[0 : T2OWN_ROWS - OWN, :])

                # ---------------- phase B: layer-1 edge phase ----------------
                if True:
                    # all windows' er vectors in one strided DMA
                    er_all = pbw.tile([P, NWIN, 4], dt.bfloat16, tag="erall")
                    nc.sync.dma_start(
                        out=er_all[:],
                        in_=t1tab[0 : NWIN * P, 264:272].bitcast(dt.bfloat16).rearrange("(w p) c -> p w c", p=P),
                    )
                    bst = {}
                    bwin = {"psum": None}
                    erpBs = {}

                    def b_stage0(sc):
                        if sc % 8 == 0:
                            erpBs[sc // 8] = ps8.tile([P, 512], dt.float32, space="PSUM", tag="erpB", bufs=1, name="erpB")
                        g = pb.tile([P, K, ROW1], dt.float8e4, tag="g", name="g")
                        msg = pb.tile([P, K, 260], dt.bfloat16, tag="msg", name="msg")
                        for q in range(KP // 1024):
                            nc.gpsimd.dma_gather(
                                g[:, 8 * q : 8 * (q + 1), :], t1tab[:, :],
                                gi1[:, sc * IW + 64 * q : sc * IW + 64 * (q + 1)],
                                1024, 1024, ROW1, queue_num=0 if single else (2 * sc + q) % 4)
                        ind_b = pb.tile([P, KP], dt.bfloat16, tag="ind", name="ind_b")
                        nc.scalar.dma_start(out=ind_b[:], in_=indp[:, sc * KP : (sc + 1) * KP])
                        bst[sc] = (g, msg, ind_b)

                    def b_stage1(sc):
                        g, msg, ind_b = bst[sc]
                        er_psum = erpBs[sc // 8][:, (sc % 8) * 64 : (sc % 8) * 64 + K * 4]
                        for j in range(K):
                            w = chunk_win[sc * K + j]
                            nc.tensor.matmul(
                                er_psum[:, j * 4 : (j + 1) * 4],
                                lhsT=ind_b[:, j * P : (j + 1) * P], rhs=er_all[:, w, :],
                                start=True, stop=True,
                            )
                        att = pb.tile([P, K, 4], dt.float32, tag="att", name="att")
                        nc.vector.tensor_tensor(
                            out=att[:], in0=g[:, :, 256:264].bitcast(dt.bfloat16),
                            in1=er_psum[:].rearrange("p (c h) -> p c h", c=K),
                            op=mybir.AluOpType.add,
                        )
                        att2 = pb.tile([P, K, 4], dt.float32, tag="att2", name="att2")
                        nc.vector.tensor_scalar_mul(att2[:], att[:], NEG)
                        nc.vector.tensor_tensor(out=att[:], in0=att[:], in1=att2[:], op=mybir.AluOpType.max)
                        nc.scalar.activation(msg[:, :, 256:260], att[:], mybir.ActivationFunctionType.Exp)
                        nc.vector.tensor_tensor(
                            out=msg[:, :, 0:HD].rearrange("p c (h d) -> p c h d", h=H),
                            in0=g[:, :, 0:HD].rearrange("p c (h d) -> p c h d", h=H),
                            in1=msg[:, :, 256:260, None].broadcast_to([P, K, 4, D]),
                            op=mybir.AluOpType.mult,
                        )

                    def b_stage2(sc):
                        g, msg, ind_b = bst.pop(sc)
                        for j in range(K):
                            ci = sc * K + j
                            w = chunk_win[ci]
                            if ci == win_first[w]:
                                bwin["psum"] = ps8.tile([P, 260], dt.float32, space="PSUM", tag="accB", bufs=2, name="accB")
                            win_psum = bwin["psum"]
                            nc.tensor.matmul(
                                win_psum[:],
                                lhsT=indt_tiles[ci][:],
                                rhs=msg[:, j, 0:260],
                                start=(ci == win_first[w]),
                                stop=(ci == win_last[w]),
                            )
                            if ci == win_last[w]:
                                m = WIN_SIZES[w]
                                den = pbfin.tile([P, 4], dt.float32, tag="den", name="den")
                                nc.vector.tensor_scalar_max(den[:], win_psum[:, 256:260], 1e-9)
                                rec = pbfin.tile([P, 4], dt.float32, tag="rec", name="rec")
                                nc.vector.reciprocal(rec[:], den[:])
                                h_sb = pbfin.tile([P, HD], dt.float32, tag="hsb", name="h_sb")
                                nc.vector.tensor_tensor(
                                    out=h_sb[:].rearrange("p (h d) -> p h d", h=H),
                                    in0=win_psum[:, 0:HD].rearrange("p (h d) -> p h d", h=H),
                                    in1=rec[:, :, None].broadcast_to([P, H, D]),
                                    op=mybir.AluOpType.mult,
                                )
                                hneg = pbfin.tile([P, HD], dt.float32, tag="hneg", name="hneg")
                                nc.vector.tensor_scalar_min(hneg[:], h_sb[:], 0.0)
                                hexp = pbfin.tile([P, HD], dt.float32, tag="hexp", name="hexp")
                                nc.scalar.activation(hexp[:], hneg[:], mybir.ActivationFunctionType.Exp)
                                nc.vector.tensor_scalar_max(h_sb[:], h_sb[:], 0.0)
                                nc.vector.tensor_tensor(out=h_sb[:], in0=h_sb[:], in1=hexp[:], op=mybir.AluOpType.add)
                                nc.vector.tensor_scalar_add(h_sb[:], h_sb[:], -1.0)
                                hT = pbfin.tile([P, 2, P], dt.bfloat16, tag="hT", name="hT")
                                for half in range(2):
                                    tp = ps8.tile([P, P], dt.float32, space="PSUM", tag="fin", bufs=1, name="tp")
                                    nc.tensor.transpose(out=tp[:, 0:m], in_=h_sb[0:m, half * P : (half + 1) * P], identity=ident[0:m, 0:m])
                                    nc.vector.tensor_copy(hT[:, half, 0:m], tp[:, 0:m])
                                t2ps_full = ps8.tile([P, P], dt.float32, space="PSUM", tag="fin", bufs=1, name="t2ps")
                                t2ps = t2ps_full[:, 0:T2W]
                                nc.tensor.matmul(t2ps[0:m, :], lhsT=hT[:, 0, 0:m], rhs=w2sb[:, 0, :], start=True, stop=False)
                                nc.tensor.matmul(t2ps[0:m, :], lhsT=hT[:, 1, 0:m], rhs=w2sb[:, 1, :], start=False, stop=True)
                                t2row = pbfin.tile([P, T2W], dt.bfloat16, tag="t2row", name="t2row")
                                nc.vector.tensor_copy(t2row[0:m, :], t2ps[0:m, :])
                                nc.sync.dma_start(out=t2own[w * P : w * P + m, 0:T2W], in_=t2row[0:m, :])

                    for t in range(nsc + 2):
                        if t < nsc:
                            b_stage0(t)
                        if 1 <= t < nsc + 1:
                            b_stage1(t - 1)
                        if t >= 2:
                            b_stage2(t - 2)

                if upto < 3:
                    continue
                # ---------------- phase C: allgather T2 ----------------
                t2own_bounce = dramp.tile([OWN, ROW2], dt.bfloat16, tag=f"t2b{r % 2}")
                nc.gpsimd.dma_start(out=t2own_bounce[:], in_=t2own[0:OWN, :])
                if single:
                    # analysis-only stand-in for the collective (TimelineSim
                    # cannot model collectives): keep the dataflow deps
                    nc.gpsimd.dma_start(out=t2all[0:OWN, :], in_=t2own_bounce[:])
                else:
                    nc.gpsimd.collective_compute(
                        "AllGather",
                        mybir.AluOpType.bypass,
                        replica_groups=[list(range(NCORES))],
                        ins=[t2own_bounce.opt()],
                        outs=[t2all.opt()],
                    )

                if upto < 4:
                    continue
                # ---------------- phase D: layer-2 edge phase ----------------
                if True:
                    er2_all = pdw.tile([P, NWIN, 1], dt.bfloat16, tag="er2all")
                    nc.sync.dma_start(
                        out=er2_all[:],
                        in_=t2own[0 : NWIN * P, 48:49].rearrange("(w p) c -> p w c", p=P),
                    )
                    erpD = ps8.tile([P, 512], dt.float32, space="PSUM", tag="erpD", bufs=1)
                    accD = ps8.tile([P, 512], dt.float32, space="PSUM", tag="accD", bufs=1)
                    dst_tiles = {}
                    dwin = {"psum": None}

                    def d_stage0(sc):
                        g2 = pd.tile([P, K, ROW2], dt.bfloat16, tag="g2", name="g2")
                        for q in range(KP // 1024):
                            nc.gpsimd.dma_gather(
                                g2[:, 8 * q : 8 * (q + 1), :], t2all[:, :],
                                gi2[:, sc * IW + 64 * q : sc * IW + 64 * (q + 1)],
                                1024, 1024, ROW2, queue_num=0 if single else (2 * sc + q) % 4)
                        ind_b = pd.tile([P, KP], dt.bfloat16, tag="ind2", name="ind_b2")
                        nc.scalar.dma_start(out=ind_b[:], in_=indp[:, sc * KP : (sc + 1) * KP])
                        dst_tiles[sc] = (g2, ind_b)

                    def d_stage1(sc):
                        g2, ind_b = dst_tiles[sc]
                        er_psum2 = erpD[:, sc * K : (sc + 1) * K]
                        for j in range(K):
                            w = chunk_win[sc * K + j]
                            nc.tensor.matmul(
                                er_psum2[:, j : j + 1],
                                lhsT=ind_b[:, j * P : (j + 1) * P], rhs=er2_all[:, w, :],
                                start=True, stop=True,
                            )
                        att = pd.tile([P, K], dt.float32, tag="attl2", name="att")
                        nc.vector.tensor_tensor(
                            out=att[:, :, None], in0=g2[:, :, 47:48], in1=er_psum2[:, :, None],
                            op=mybir.AluOpType.add,
                        )
                        att2 = pd.tile([P, K], dt.float32, tag="attl2b", name="att2")
                        nc.vector.tensor_scalar_mul(att2[:], att[:], NEG)
                        nc.vector.tensor_tensor(out=att[:], in0=att[:], in1=att2[:], op=mybir.AluOpType.max)
                        nc.scalar.activation(g2[:, :, 49:50], att[:, :, None], mybir.ActivationFunctionType.Exp)
                        nc.vector.tensor_tensor(
                            out=g2[:, :, 0:48],
                            in0=g2[:, :, 0:48],
                            in1=g2[:, :, 49:50].broadcast_to([P, K, 48]),
                            op=mybir.AluOpType.mult,
                        )

                    def d_stage2(sc, last):
                        g2, ind_b = dst_tiles.pop(sc)
                        for j in range(K):
                            ci = sc * K + j
                            w = chunk_win[ci]
                            if ci == win_first[w]:
                                dwin["psum"] = accD[:, (w % 4) * P : (w % 4) * P + 50]
                            win_psum2 = dwin["psum"]
                            nc.tensor.matmul(
                                win_psum2[:],
                                lhsT=indt_tiles[ci][:],
                                rhs=g2[:, j, 0:50],
                                start=(ci == win_first[w]),
                                stop=(ci == win_last[w]),
                            )
                            if ci == win_last[w]:
                                m = WIN_SIZES[w]
                                den = pdfin.tile([P, 1], dt.float32, tag="den2", name="den")
                                nc.vector.tensor_scalar_max(den[:], win_psum2[:, 49:50], 1e-9)
                                rec = pdfin.tile([P, 1], dt.float32, tag="rec2", name="rec")
                                nc.vector.reciprocal(rec[:], den[:])
                                logit = pdfin.tile([P, OUTF], dt.float32, tag="logit", name="logit")
                                nc.vector.tensor_scalar(
                                    out=logit[:], in0=win_psum2[:, 0:OUTF],
                                    scalar1=rec[:, 0:1], scalar2=None,
                                    op0=mybir.AluOpType.mult,
                                )
                                mx = pdfin.tile([P, 1], dt.float32, tag="mx", name="mx")
                                nc.vector.tensor_reduce(mx[:], logit[:], mybir.AxisListType.X, mybir.AluOpType.max)
                                nc.vector.tensor_scalar(
                                    out=logit[:], in0=logit[:], scalar1=mx[:, 0:1], scalar2=None,
                                    op0=mybir.AluOpType.subtract,
                                )
                                exps = pdfin.tile([P, OUTF], dt.float32, tag="exps", name="exps")
                                se = pdfin.tile([P, 1], dt.float32, tag="se", name="se")
                                nc.scalar.activation(exps[:], logit[:], mybir.ActivationFunctionType.Exp, accum_out=se[:])
                                lse = pdfin.tile([P, 1], dt.float32, tag="lse", name="lse")
                                nc.scalar.activation(lse[:], se[:], mybir.ActivationFunctionType.Ln)
                                nc.vector.tensor_scalar(
                                    out=logit[:], in0=logit[:], scalar1=lse[:, 0:1], scalar2=None,
                                    op0=mybir.AluOpType.subtract,
                                )
                                if last:
                                    nc.sync.dma_start(out=outp[w * P : w * P + m, :], in_=logit[0:m, :])

                    a_next = a_closures(r + 1) if (r + 1 < reps and upto >= 2) else []
                    ai = 0
                    for t in range(nsc + 2):
                        if t < nsc:
                            d_stage0(t)
                        if 1 <= t < nsc + 1:
                            d_stage1(t - 1)
                        if t >= 2:
                            d_stage2(t - 2, last)
                        while ai * (nsc + 2) < len(a_next) * (t + 1):
                            a_next[ai]()
                            ai += 1
    nc.compile()
    return nc


# ----------------------------------------------------------------------------
# host entry
# ----------------------------------------------------------------------------

def _host_inputs(features, src, dst, W1, al1, ar1, W2, al2, ar2):
    feats = np.asarray(features, np.float32)
    W1 = np.asarray(W1, np.float32)
    W2 = np.asarray(W2, np.float32)
    al1 = np.asarray(al1, np.float32)
    ar1 = np.asarray(ar1, np.float32)
    al2 = np.asarray(al2, np.float32)
    ar2 = np.asarray(ar2, np.float32)

    Wl1 = np.stack([W1[:, h * D : (h + 1) * D] @ al1[h] for h in range(H)], axis=1)
    Wr1 = np.stack([W1[:, h * D : (h + 1) * D] @ ar1[h] for h in range(H)], axis=1)
    W1p = np.concatenate([W1, Wl1, Wr1], axis=1).astype(BF16)          # [256, 264]
    Wl2 = (W2 @ al2[0])[:, None]
    Wr2 = (W2 @ ar2[0])[:, None]
    W2p = np.concatenate([W2, Wl2, Wr2, np.zeros((HD, 3), np.float32)], axis=1).astype(BF16)  # [256, 52]
    W1P = np.ascontiguousarray(W1p.reshape(2, P, 264).transpose(1, 0, 2))
    W2P = np.ascontiguousarray(W2p.reshape(2, P, T2W).transpose(1, 0, 2))

    graph, cw, nchunk, nsc, chunk_win = _prep_graph(src, dst)
    featT = np.ascontiguousarray(feats.T)                               # [256, N]
    in_maps = []
    for c in range(NCORES):
        featTl = np.roll(featT, -OWN * c, axis=1)                       # local node order
        featPc = np.ascontiguousarray(
            featTl.reshape(2, P, N).transpose(1, 2, 0)).astype(BF16)    # [128, N, 2]
        in_maps.append(dict(
            featP=featPc,
            W1P=W1P, W2P=W2P,
            gidx1=graph[c]["gidx1"], gidx2=graph[c]["gidx2"],
            ind=graph[c]["ind"], indt=graph[c]["indt"],
        ))
    return in_maps, nchunk, nsc, chunk_win


def kernel(features, src, dst, W1, al1, ar1, W2, al2, ar2):
    from concourse.bass_utils import run_bass_kernel_spmd

    in_maps, nchunk, nsc, chunk_win = _host_inputs(
        features, src, dst, W1, al1, ar1, W2, al2, ar2)
    key = (nchunk, nsc, tuple(chunk_win))
    if key not in _CACHE:
        _CACHE[key] = build_program(nchunk, nsc, chunk_win, reps=1)
    nc = _CACHE[key]
    res = run_bass_kernel_spmd(nc, in_maps, core_ids=list(range(NCORES)))
    return np.concatenate([res.results[c]["out"] for c in range(NCORES)], axis=0)


# revision 34
# speedup vs baseline: 1.5305x; 1.2961x over previous
"""Trainium2 Bass kernel for 2-layer GAT (nn_GAT_34832184770812).

Strategy (8 NeuronCores, dst-node sharded):
- Each core owns 1250 dst nodes; node ids are rotated per core so own nodes
  are local rows 0:1250 (keeps the SPMD program identical across cores).
- Phase A: T1 = features @ [W1 | W1@al1 | W1@ar1] (bf16, replicated) ->
  DRAM gather table T1tab[N, 384] (feat 256 | el 256:260 | er 260:264 | pad).
- Phase B (layer-1 edge phase): edges sorted by dst window (128 dst rows per
  window, padded to 128-edge chunks, chunk counts uniform across cores).
  Per 1024-edge superchunk: one dma_gather of src rows (Pool/SWDGE — no
  HWDGE); one batched load of the one-hot dst indicators for all 8 chunks;
  er per edge via indicator matmuls; e = lrelu(el+er), ex = exp(e) (softmax
  is shift-invariant, exponents are O(1)); messages scaled by ex; segment
  sum over dst via indicator-transpose matmul (IndT resident in SBUF), with
  ex as extra columns producing softmax denominators in the same psum.
- Window finalize: normalize, ELU, transpose (PE), T2own = h @ W2p.
- AllGather T2own (bf16, 52-wide) -> expand to 256B rows -> T2all gather
  table.
- Phase D (layer-2 edge phase): same structure, 1 head, 47 feats.
- log_softmax per window, output [1250, 47] f32 per core, host concat.

DMA-issue budget drives the design: every HWDGE dma_start costs ~630ns on a
device shared by all queues, so host-constant tables (indices, weights,
indicator transposes) are loaded once outside the rep loop, per-chunk loads
are batched per superchunk, and gathers/bulk copies ride the Pool-engine
SWDGE path which bypasses HWDGE entirely.
"""

import numpy as np
import ml_dtypes

BF16 = ml_dtypes.bfloat16
FP8 = ml_dtypes.float8_e4m3fn

# problem constants (hardcoded per contract)
N = 10000
E = 320000
IN_FEATS = 256
H = 4
D = 64
HD = 256
OUTF = 47
NEG = 0.2
NCORES = 8
OWN = N // NCORES          # 1250
P = 128
NWIN = (OWN + P - 1) // P  # 10 windows (last has 98 nodes)
WIN_SIZES = [min(P, OWN - P * w) for w in range(NWIN)]
K = 16                     # chunks per superchunk
KP = K * P                 # edges per superchunk
IW = KP // 16              # idx cols per superchunk (wrapped 16-row layout)
ROW1 = 512                 # T1 gather row (fp8): feat 0:256 | el bf16 @bytes 256:264 | er bf16 @bytes 264:272 | pad
ROW2 = 128                 # T2 gather row (bf16): feat 0:47 | el2 47 | er2 48 | ex2 49 | pad
T2W = 52                   # t2own row width (47 feat + el + er + ex + pad to 52)
T2OWN_ROWS = NWIN * P      # 1280 (rows 1250:1280 zeroed)

_CACHE = {}


# ----------------------------------------------------------------------------
# host-side graph preprocessing
# ----------------------------------------------------------------------------

def _prep_graph(src, dst):
    """Per-core edge partition, window sort, uniform padding, one-hots."""
    src = np.asarray(src).astype(np.int64)
    dst = np.asarray(dst).astype(np.int64)
    core_of = dst // OWN
    per_core = []
    for c in range(NCORES):
        sel = np.nonzero(core_of == c)[0]
        dl = dst[sel] - OWN * c                       # local dst in [0, OWN)
        sl = (src[sel] - OWN * c) % N                 # local src
        # sort by (window, src) — src-ascending gathers get HBM locality;
        # dstrow within a chunk is free (one-hots encode it)
        order = np.lexsort((sl, dl // P))
        per_core.append((dl[order], sl[order], src[sel][order]))

    # uniform chunks per window across cores
    cw = []
    bounds = []
    for c in range(NCORES):
        dl = per_core[c][0]
        b = np.searchsorted(dl, [P * w for w in range(NWIN + 1)])
        bounds.append(b)
    for w in range(NWIN):
        mx = max(bounds[c][w + 1] - bounds[c][w] for c in range(NCORES))
        cw.append((int(mx) + P - 1) // P)
    nchunk = sum(cw)
    nsc = (nchunk + K - 1) // K
    pad_chunks = nsc * K - nchunk
    cw[-1] += pad_chunks
    nchunk = nsc * K

    chunk_win = []
    for w in range(NWIN):
        chunk_win += [w] * cw[w]

    ES = nchunk * P
    out = []
    for c in range(NCORES):
        dl, sl, sg = per_core[c]
        b = bounds[c]
        src_loc = np.zeros(ES, np.int16)
        src_glb = np.zeros(ES, np.int16)
        dstrow = np.full(ES, -1, np.int32)            # -1 = dummy
        pos = 0
        for w in range(NWIN):
            e0, e1 = b[w], b[w + 1]
            n = e1 - e0
            src_loc[pos : pos + n] = sl[e0:e1]
            src_glb[pos : pos + n] = sg[e0:e1]
            dstrow[pos : pos + n] = dl[e0:e1] - P * w
            pos += cw[w] * P
        # one-hot indicators; ind is partition-major [dstrow, chunk*128+e] so
        # a whole superchunk loads as one [128, 1024] DMA
        ind = np.zeros((P, ES), ml_dtypes.float8_e4m3fn)
        indt = np.zeros((ES, P), BF16)                # [chunk*128 + e, dstrow]
        ch = np.arange(ES) // P
        e_in = np.arange(ES) % P
        real = dstrow >= 0
        r = np.nonzero(real)[0]
        ind[dstrow[r], ch[r] * P + e_in[r]] = 1
        indt[ch[r] * P + e_in[r], dstrow[r]] = 1
        # dma_gather wrapped idx layout, horizontally concatenated per
        # superchunk: [128, nsc*64]
        def wrap(ids):
            lay = np.zeros((P, nsc * IW), np.int16)
            for sc in range(nsc):
                blk = ids[sc * KP : (sc + 1) * KP]
                wr = np.zeros((16, IW), np.int16)
                kk = np.arange(KP)
                wr[kk % 16, kk // 16] = blk
                lay[:, sc * IW : (sc + 1) * IW] = np.tile(wr, (8, 1))
            return lay
        out.append(dict(gidx1=wrap(src_loc), gidx2=wrap(src_glb), ind=ind, indt=indt))
    return out, cw, nchunk, nsc, chunk_win


# ----------------------------------------------------------------------------
# program build
# ----------------------------------------------------------------------------

def build_program(nchunk, nsc, chunk_win, reps=1, single=False, upto=4, xray=0):
    import concourse.tile as tile
    from concourse import bacc, mybir
    from concourse.masks import make_identity

    NT = (N + P - 1) // P                              # 79 node tiles
    # chunk boundaries: first/last chunk of each window
    win_first = {}
    win_last = {}
    for ci, w in enumerate(chunk_win):
        if w not in win_first:
            win_first[w] = ci
        win_last[w] = ci

    nc = bacc.Bacc("TRN2", target_bir_lowering=False, debug=False,
                   num_devices=1 if single else NCORES, num_swdge_queues=4)
    dt = mybir.dt
    featP = nc.declare_dram_parameter("featP", [P, N, 2], dt.bfloat16, isOutput=False)
    W1P = nc.declare_dram_parameter("W1P", [P, 2, 264], dt.bfloat16, isOutput=False)
    W2P = nc.declare_dram_parameter("W2P", [P, 2, T2W], dt.bfloat16, isOutput=False)
    gidx1 = nc.declare_dram_parameter("gidx1", [P, nsc * IW], dt.int16, isOutput=False)
    gidx2 = nc.declare_dram_parameter("gidx2", [P, nsc * IW], dt.int16, isOutput=False)
    indp = nc.declare_dram_parameter("ind", [P, nchunk * P], dt.float8e4, isOutput=False)
    indtp = nc.declare_dram_parameter("indt", [nchunk * P, P], dt.bfloat16, isOutput=False)
    outp = nc.declare_dram_parameter("out", [OWN, OUTF], dt.float32, isOutput=True)

    t1tabs = [nc.dram_tensor(f"t1tab{i}", [N, ROW1], dt.float8e4) for i in range(2)]
    t2owns = [nc.dram_tensor(f"t2own{i}", [T2OWN_ROWS, ROW2], dt.bfloat16) for i in range(2)]

    with tile.TileContext(nc) as tc:
        with (
            tc.tile_pool(name="const", bufs=1) as constp,
            tc.tile_pool(name="res", bufs=max(1, nchunk)) as respool,
            tc.tile_pool(name="dram", bufs=1, space="DRAM") as dramp,
        ):
            ident = constp.tile([P, P], dt.float32)
            make_identity(nc, ident[:])
            zero52 = constp.tile([P, T2W], dt.bfloat16)
            nc.vector.memset(zero52[:], 0)

            # resident constants: weights, gather indices, IndT tiles
            w1sb = constp.tile([P, 2, 264], dt.bfloat16)
            nc.sync.dma_start(out=w1sb[:], in_=W1P[:, :, :])
            w2sb = constp.tile([P, 2, T2W], dt.bfloat16)
            nc.sync.dma_start(out=w2sb[:], in_=W2P[:, :, :])
            gi1 = constp.tile([P, nsc * IW], dt.int16)
            nc.sync.dma_start(out=gi1[:], in_=gidx1[:, :])
            gi2 = constp.tile([P, nsc * IW], dt.int16)
            nc.sync.dma_start(out=gi2[:], in_=gidx2[:, :])
            indt_tiles = []
            for ci in range(nchunk):
                t = respool.tile([P, P], dt.bfloat16, tag="res")
                nc.sync.dma_start(out=t[:], in_=indtp[ci * P : (ci + 1) * P, :])
                indt_tiles.append(t)

            t2all0 = dramp.tile([N, ROW2], dt.bfloat16, tag="t2all0")
            t2all1 = dramp.tile([N, ROW2], dt.bfloat16, tag="t2all1")
            t2alls = [t2all0, t2all1]

            with (
                tc.tile_pool(name="pa", bufs=4) as pa,
                tc.tile_pool(name="pb", bufs=3) as pb,
                tc.tile_pool(name="pbw", bufs=2) as pbw,
                tc.tile_pool(name="pbfin", bufs=2) as pbfin,
                tc.tile_pool(name="pd", bufs=3) as pd,
                tc.tile_pool(name="pdw", bufs=2) as pdw,
                tc.tile_pool(name="pdfin", bufs=2) as pdfin,
                tc.tile_pool(name="ps8", bufs=1, space="PSUM") as ps8,
            ):
              def a_closures(rr):
                """Phase A for rep rr as per-iteration closures (interleavable)."""
                t1 = t1tabs[rr % 2]
                NB = N // (2 * P)                      # 39 full pairs + tail
                fns = []

                def full_iter(nb):
                    def f():
                        lt = pa.tile([P, 2 * P, 2], dt.bfloat16, tag="lt", name="lt")
                        nc.sync.dma_start(out=lt[:], in_=featP[:, nb * 2 * P : (nb + 1) * 2 * P, :])
                        row = pa.tile([P, 2, 272], dt.float8e4, tag="row", name="row")
                        for a in range(2):
                            ps = ps8.tile([P, 264], dt.float32, space="PSUM", tag="paps", bufs=2, name="ps")
                            sl = slice(a * P, (a + 1) * P)
                            nc.tensor.matmul(ps[:], lhsT=lt[:, sl, 0], rhs=w1sb[:, 0, :], start=True, stop=False)
                            nc.tensor.matmul(ps[:], lhsT=lt[:, sl, 1], rhs=w1sb[:, 1, :], start=False, stop=True)
                            nc.vector.tensor_copy(row[:, a, 0:256], ps[:, 0:256])
                            nc.vector.tensor_copy(row[:, a, 256:272].bitcast(dt.bfloat16), ps[:, 256:264])
                        nc.scalar.dma_start(
                            out=t1[nb * 2 * P : (nb + 1) * 2 * P, 0:272].rearrange("(a p) c -> p a c", a=2),
                            in_=row[:],
                        )
                    return f

                def tail_iter():
                    mt = N - NB * 2 * P
                    ltt = pa.tile([P, mt, 2], dt.bfloat16, tag="ltt", name="ltt")
                    nc.sync.dma_start(out=ltt[:], in_=featP[:, NB * 2 * P : N, :])
                    ps = ps8.tile([P, 264], dt.float32, space="PSUM", tag="paps", bufs=2, name="ps")
                    nc.tensor.matmul(ps[0:mt, :], lhsT=ltt[:, :, 0], rhs=w1sb[:, 0, :], start=True, stop=False)
                    nc.tensor.matmul(ps[0:mt, :], lhsT=ltt[:, :, 1], rhs=w1sb[:, 1, :], start=False, stop=True)
                    rowt = pa.tile([P, 272], dt.float8e4, tag="rowt", name="rowt")
                    nc.vector.tensor_copy(rowt[0:mt, 0:256], ps[0:mt, 0:256])
                    nc.vector.tensor_copy(rowt[0:mt, 256:272].bitcast(dt.bfloat16), ps[0:mt, 256:264])
                    nc.scalar.dma_start(out=t1[NB * 2 * P : N, 0:272], in_=rowt[0:mt, :])

                for nb in range(NB):
                    fns.append(full_iter(nb))
                fns.append(tail_iter)
                return fns

              for r in range(reps):
                last = r == reps - 1
                t1tab = t1tabs[r % 2]
                t2own = t2owns[r % 2]
                t2all = t2alls[r % 2]
                # ---------------- phase A ----------------
                # rep 0 runs inline; A(r+1) is emitted interleaved into D(r)
                if r == 0:
                    for f in a_closures(0):
                        f()

                if upto < 2:
                    continue
                # zero t2own pad rows once
                nc.gpsimd.dma_start(out=t2own[OWN:T2OWN_ROWS, 0:T2W], in_=zero52[0 : T2OWN_ROWS - OWN, :])

                # ---------------- phase B: layer-1 edge phase ----------------
                if True:
                    # all windows' er vectors in one strided DMA
                    er_all_bf = pbw.tile([P, NWIN, 4], dt.bfloat16, tag="erallbf")
                    nc.sync.dma_start(
                        out=er_all_bf[:],
                        in_=t1tab[0 : NWIN * P, 264:272].bitcast(dt.bfloat16).rearrange("(w p) c -> p w c", p=P),
                    )
                    er_all = pbw.tile([P, NWIN, 4], dt.float8e4, tag="erall")
                    nc.vector.tensor_copy(er_all[:], er_all_bf[:])
                    bst = {}
                    bwin = {"psum": None}
                    erpBs = {}

                    def b_stage0(sc):
                        if sc % 8 == 0:
                            erpBs[sc // 8] = ps8.tile([P, 512], dt.float32, space="PSUM", tag="erpB", bufs=1, name="erpB")
                        g = pb.tile([P, K, ROW1], dt.float8e4, tag="g", name="g")
                        msg = pb.tile([P, K, 260], dt.bfloat16, tag="msg", name="msg")
                        for q in range(KP // 1024):
                            nc.gpsimd.dma_gather(
                                g[:, 8 * q : 8 * (q + 1), :], t1tab[:, :],
                                gi1[:, sc * IW + 64 * q : sc * IW + 64 * (q + 1)],
                                1024, 1024, ROW1, queue_num=0 if single else (2 * sc + q) % 4)
                        ind_b = pb.tile([P, KP], dt.float8e4, tag="ind", name="ind_b")
                        nc.scalar.dma_start(out=ind_b[:], in_=indp[:, sc * KP : (sc + 1) * KP])
                        bst[sc] = (g, msg, ind_b)

                    def b_stage1(sc):
                        g, msg, ind_b = bst[sc]
                        er_psum = erpBs[sc // 8][:, (sc % 8) * 64 : (sc % 8) * 64 + K * 4]
                        for j in range(K):
                            w = chunk_win[sc * K + j]
                            nc.tensor.matmul(
                                er_psum[:, j * 4 : (j + 1) * 4],
                                lhsT=ind_b[:, j * P : (j + 1) * P], rhs=er_all[:, w, :],
                                start=True, stop=True,
                            )
                        att = pb.tile([P, K, 4], dt.float32, tag="att", name="att")
                        nc.vector.tensor_tensor(
                            out=att[:], in0=g[:, :, 256:264].bitcast(dt.bfloat16),
                            in1=er_psum[:].rearrange("p (c h) -> p c h", c=K),
                            op=mybir.AluOpType.add,
                        )
                        att2 = pb.tile([P, K, 4], dt.float32, tag="att2", name="att2")
                        nc.vector.tensor_scalar_mul(att2[:], att[:], NEG)
                        nc.vector.tensor_tensor(out=att[:], in0=att[:], in1=att2[:], op=mybir.AluOpType.max)
                        nc.scalar.activation(msg[:, :, 256:260], att[:], mybir.ActivationFunctionType.Exp)
                        nc.vector.tensor_tensor(
                            out=msg[:, :, 0:HD].rearrange("p c (h d) -> p c h d", h=H),
                            in0=g[:, :, 0:HD].rearrange("p c (h d) -> p c h d", h=H),
                            in1=msg[:, :, 256:260, None].broadcast_to([P, K, 4, D]),
                            op=mybir.AluOpType.mult,
                        )

                    def b_stage2(sc):
                        g, msg, ind_b = bst.pop(sc)
                        for j in range(K):
                            ci = sc * K + j
                            w = chunk_win[ci]
                            if ci == win_first[w]:
                                bwin["psum"] = ps8.tile([P, 260], dt.float32, space="PSUM", tag="accB", bufs=2, name="accB")
                            win_psum = bwin["psum"]
                            nc.tensor.matmul(
                                win_psum[:],
                                lhsT=indt_tiles[ci][:],
                                rhs=msg[:, j, 0:260],
                                start=(ci == win_first[w]),
                                stop=(ci == win_last[w]),
                            )
                            if ci == win_last[w]:
                                m = WIN_SIZES[w]
                                den = pbfin.tile([P, 4], dt.float32, tag="den", name="den")
                                nc.vector.tensor_scalar_max(den[:], win_psum[:, 256:260], 1e-9)
                                rec = pbfin.tile([P, 4], dt.float32, tag="rec", name="rec")
                                nc.vector.reciprocal(rec[:], den[:])
                                h_sb = pbfin.tile([P, HD], dt.float32, tag="hsb", name="h_sb")
                                nc.vector.tensor_tensor(
                                    out=h_sb[:].rearrange("p (h d) -> p h d", h=H),
                                    in0=win_psum[:, 0:HD].rearrange("p (h d) -> p h d", h=H),
                                    in1=rec[:, :, None].broadcast_to([P, H, D]),
                                    op=mybir.AluOpType.mult,
                                )
                                hneg = pbfin.tile([P, HD], dt.float32, tag="hneg", name="hneg")
                                nc.vector.tensor_scalar_min(hneg[:], h_sb[:], 0.0)
                                hexp = pbfin.tile([P, HD], dt.float32, tag="hexp", name="hexp")
                                nc.scalar.activation(hexp[:], hneg[:], mybir.ActivationFunctionType.Exp)
                                nc.vector.tensor_scalar_max(h_sb[:], h_sb[:], 0.0)
                                nc.vector.tensor_tensor(out=h_sb[:], in0=h_sb[:], in1=hexp[:], op=mybir.AluOpType.add)
                                nc.vector.tensor_scalar_add(h_sb[:], h_sb[:], -1.0)
                                hT = pbfin.tile([P, 2, P], dt.bfloat16, tag="hT", name="hT")
                                for half in range(2):
                                    tp = ps8.tile([P, P], dt.float32, space="PSUM", tag="fin", bufs=1, name="tp")
                                    nc.tensor.transpose(out=tp[:, 0:m], in_=h_sb[0:m, half * P : (half + 1) * P], identity=ident[0:m, 0:m])
                                    nc.vector.tensor_copy(hT[:, half, 0:m], tp[:, 0:m])
                                t2ps_full = ps8.tile([P, P], dt.float32, space="PSUM", tag="fin", bufs=1, name="t2ps")
                                t2ps = t2ps_full[:, 0:T2W]
                                nc.tensor.matmul(t2ps[0:m, :], lhsT=hT[:, 0, 0:m], rhs=w2sb[:, 0, :], start=True, stop=False)
                                nc.tensor.matmul(t2ps[0:m, :], lhsT=hT[:, 1, 0:m], rhs=w2sb[:, 1, :], start=False, stop=True)
                                t2row = pbfin.tile([P, T2W], dt.bfloat16, tag="t2row", name="t2row")
                                nc.vector.tensor_copy(t2row[0:m, :], t2ps[0:m, :])
                                nc.sync.dma_start(out=t2own[w * P : w * P + m, 0:T2W], in_=t2row[0:m, :])

                    for t in range(nsc + 2):
                        if t < nsc:
                            b_stage0(t)
                        if 1 <= t < nsc + 1:
                            b_stage1(t - 1)
                        if t >= 2:
                            b_stage2(t - 2)

                if upto < 3:
                    continue
                # ---------------- phase C: allgather T2 ----------------
                t2own_bounce = dramp.tile([OWN, ROW2], dt.bfloat16, tag=f"t2b{r % 2}")
                nc.gpsimd.dma_start(out=t2own_bounce[:], in_=t2own[0:OWN, :])
                if single:
                    # analysis-only stand-in for the collective (TimelineSim
                    # cannot model collectives): keep the dataflow deps
                    nc.gpsimd.dma_start(out=t2all[0:OWN, :], in_=t2own_bounce[:])
                else:
                    nc.gpsimd.collective_compute(
                        "AllGather",
                        mybir.AluOpType.bypass,
                        replica_groups=[list(range(NCORES))],
                        ins=[t2own_bounce.opt()],
                        outs=[t2all.opt()],
                    )

                if upto < 4:
                    continue
                # ---------------- phase D: layer-2 edge phase ----------------
                if True:
                    er2_all_bf = pdw.tile([P, NWIN, 1], dt.bfloat16, tag="er2allbf")
                    nc.sync.dma_start(
                        out=er2_all_bf[:],
                        in_=t2own[0 : NWIN * P, 48:49].rearrange("(w p) c -> p w c", p=P),
                    )
                    er2_all = pdw.tile([P, NWIN, 1], dt.float8e4, tag="er2all")
                    nc.vector.tensor_copy(er2_all[:], er2_all_bf[:])
                    erpD = ps8.tile([P, 512], dt.float32, space="PSUM", tag="erpD", bufs=1)
                    accD = ps8.tile([P, 512], dt.float32, space="PSUM", tag="accD", bufs=1)
                    dst_tiles = {}
                    dwin = {"psum": None}

                    def d_stage0(sc):
                        g2 = pd.tile([P, K, ROW2], dt.bfloat16, tag="g2", name="g2")
                        for q in range(KP // 1024):
                            nc.gpsimd.dma_gather(
                                g2[:, 8 * q : 8 * (q + 1), :], t2all[:, :],
                                gi2[:, sc * IW + 64 * q : sc * IW + 64 * (q + 1)],
                                1024, 1024, ROW2, queue_num=0 if single else (2 * sc + q) % 4)
                        ind_b = pd.tile([P, KP], dt.float8e4, tag="ind2", name="ind_b2")
                        nc.scalar.dma_start(out=ind_b[:], in_=indp[:, sc * KP : (sc + 1) * KP])
                        dst_tiles[sc] = (g2, ind_b)

                    def d_stage1(sc):
                        g2, ind_b = dst_tiles[sc]
                        er_psum2 = erpD[:, sc * K : (sc + 1) * K]
                        for j in range(K):
                            w = chunk_win[sc * K + j]
                            nc.tensor.matmul(
                                er_psum2[:, j : j + 1],
                                lhsT=ind_b[:, j * P : (j + 1) * P], rhs=er2_all[:, w, :],
                                start=True, stop=True,
                            )
                        att = pd.tile([P, K], dt.float32, tag="attl2", name="att")
                        nc.vector.tensor_tensor(
                            out=att[:, :, None], in0=g2[:, :, 47:48], in1=er_psum2[:, :, None],
                            op=mybir.AluOpType.add,
                        )
                        att2 = pd.tile([P, K], dt.float32, tag="attl2b", name="att2")
                        nc.vector.tensor_scalar_mul(att2[:], att[:], NEG)
                        nc.vector.tensor_tensor(out=att[:], in0=att[:], in1=att2[:], op=mybir.AluOpType.max)
                        nc.scalar.activation(g2[:, :, 49:50], att[:, :, None], mybir.ActivationFunctionType.Exp)
                        nc.vector.tensor_tensor(
                            out=g2[:, :, 0:48],
                            in0=g2[:, :, 0:48],
                            in1=g2[:, :, 49:50].broadcast_to([P, K, 48]),
                            op=mybir.AluOpType.mult,
                        )

                    def d_stage2(sc, last):
                        g2, ind_b = dst_tiles.pop(sc)
                        for j in range(K):
                            ci = sc * K + j
                            w = chunk_win[ci]
                            if ci == win_first[w]:
                                dwin["psum"] = accD[:, (w % 4) * P : (w % 4) * P + 50]
                            win_psum2 = dwin["psum"]
                            nc.tensor.matmul(
                                win_psum2[:],
                                lhsT=indt_tiles[ci][:],
                                rhs=g2[:, j, 0:50],
                                start=(ci == win_first[w]),
                                stop=(ci == win_last[w]),
                            )
                            if ci == win_last[w]:
                                m = WIN_SIZES[w]
                                den = pdfin.tile([P, 1], dt.float32, tag="den2", name="den")
                                nc.vector.tensor_scalar_max(den[:], win_psum2[:, 49:50], 1e-9)
                                rec = pdfin.tile([P, 1], dt.float32, tag="rec2", name="rec")
                                nc.vector.reciprocal(rec[:], den[:])
                                logit = pdfin.tile([P, OUTF], dt.float32, tag="logit", name="logit")
                                nc.vector.tensor_scalar(
                                    out=logit[:], in0=win_psum2[:, 0:OUTF],
                                    scalar1=rec[:, 0:1], scalar2=None,
                                    op0=mybir.AluOpType.mult,
                                )
                                mx = pdfin.tile([P, 1], dt.float32, tag="mx", name="mx")
                                nc.vector.tensor_reduce(mx[:], logit[:], mybir.AxisListType.X, mybir.AluOpType.max)
                                nc.vector.tensor_scalar(
                                    out=logit[:], in0=logit[:], scalar1=mx[:, 0:1], scalar2=None,
                                    op0=mybir.AluOpType.subtract,
                                )
                                exps = pdfin.tile([P, OUTF], dt.float32, tag="exps", name="exps")
                                se = pdfin.tile([P, 1], dt.float32, tag="se", name="se")
                                nc.scalar.activation(exps[:], logit[:], mybir.ActivationFunctionType.Exp, accum_out=se[:])
                                lse = pdfin.tile([P, 1], dt.float32, tag="lse", name="lse")
                                nc.scalar.activation(lse[:], se[:], mybir.ActivationFunctionType.Ln)
                                nc.vector.tensor_scalar(
                                    out=logit[:], in0=logit[:], scalar1=lse[:, 0:1], scalar2=None,
                                    op0=mybir.AluOpType.subtract,
                                )
                                if last:
                                    nc.sync.dma_start(out=outp[w * P : w * P + m, :], in_=logit[0:m, :])

                    a_next = a_closures(r + 1) if (r + 1 < reps and upto >= 2) else []
                    ai = 0
                    for t in range(nsc + 2):
                        if t < nsc:
                            d_stage0(t)
                        if 1 <= t < nsc + 1:
                            d_stage1(t - 1)
                        if t >= 2:
                            d_stage2(t - 2, last)
                        while ai * (nsc + 2) < len(a_next) * (t + 1):
                            a_next[ai]()
                            ai += 1
    nc.compile()
    return nc


# ----------------------------------------------------------------------------
# host entry
# ----------------------------------------------------------------------------

def _host_inputs(features, src, dst, W1, al1, ar1, W2, al2, ar2):
    feats = np.asarray(features, np.float32)
    W1 = np.asarray(W1, np.float32)
    W2 = np.asarray(W2, np.float32)
    al1 = np.asarray(al1, np.float32)
    ar1 = np.asarray(ar1, np.float32)
    al2 = np.asarray(al2, np.float32)
    ar2 = np.asarray(ar2, np.float32)

    Wl1 = np.stack([W1[:, h * D : (h + 1) * D] @ al1[h] for h in range(H)], axis=1)
    Wr1 = np.stack([W1[:, h * D : (h + 1) * D] @ ar1[h] for h in range(H)], axis=1)
    W1p = np.concatenate([W1, Wl1, Wr1], axis=1).astype(BF16)          # [256, 264]
    Wl2 = (W2 @ al2[0])[:, None]
    Wr2 = (W2 @ ar2[0])[:, None]
    W2p = np.concatenate([W2, Wl2, Wr2, np.zeros((HD, 3), np.float32)], axis=1).astype(BF16)  # [256, 52]
    W1P = np.ascontiguousarray(W1p.reshape(2, P, 264).transpose(1, 0, 2))
    W2P = np.ascontiguousarray(W2p.reshape(2, P, T2W).transpose(1, 0, 2))

    graph, cw, nchunk, nsc, chunk_win = _prep_graph(src, dst)
    featT = np.ascontiguousarray(feats.T)                               # [256, N]
    in_maps = []
    for c in range(NCORES):
        featTl = np.roll(featT, -OWN * c, axis=1)                       # local node order
        featPc = np.ascontiguousarray(
            featTl.reshape(2, P, N).transpose(1, 2, 0)).astype(BF16)    # [128, N, 2]
        in_maps.append(dict(
            featP=featPc,
            W1P=W1P, W2P=W2P,
            gidx1=graph[c]["gidx1"], gidx2=graph[c]["gidx2"],
            ind=graph[c]["ind"], indt=graph[c]["indt"],
        ))
    return in_maps, nchunk, nsc, chunk_win


def kernel(features, src, dst, W1, al1, ar1, W2, al2, ar2):
    from concourse.bass_utils import run_bass_kernel_spmd

    in_maps, nchunk, nsc, chunk_win = _host_inputs(
        features, src, dst, W1, al1, ar1, W2, al2, ar2)
    key = (nchunk, nsc, tuple(chunk_win))
    if key not in _CACHE:
        _CACHE[key] = build_program(nchunk, nsc, chunk_win, reps=1)
    nc = _CACHE[key]
    res = run_bass_kernel_spmd(nc, in_maps, core_ids=list(range(NCORES)))
    return np.concatenate([res.results[c]["out"] for c in range(NCORES)], axis=0)
